# revision 42
# baseline (speedup 1.0000x reference)
# Multi-head masked attention (V = Q source quirk; Wv unused) on 8 TRN2 NeuronCores.
#
# Sharding: 8 cores = 4 batches x 2 head-halves (tensor parallel). Core c
# handles batch b = c//2 and heads hh*8..hh*8+7 (hh = c%2), for ALL queries.
# Each core projects K^T and Q-natural (= V) for its own 8 heads only (no
# duplicated projection work across the pair, unlike a query split), derives
# Q^T from Q-natural via PE transposes (4x cheaper than re-projecting), and
# runs causal attention for its heads over all 2048 queries. The pair then
# exchanges normalized attention outputs per query-chunk with pairwise
# AllGather collectives (chosen over ReduceScatter: ~16us vs ~45us per op on
# this NRT), after which each core runs the output projection over all 16
# heads for ITS 512 output columns (Wo column-half + bias live in the
# per-core input data) and writes out^T[my_e, all_q] directly. The program
# is fully SPMD-uniform: head/e-column assignment is carried by input data,
# and both pair contributions come back from the AllGather in fixed device
# order, so no rank-dependent addressing exists anywhere.
#
# Layouts (per core, bf16 matmul operands, fp32 PSUM accumulation):
#   kT  [128=d-in-pair, HP=4, S]   scores lhsT  (head even: partitions 0-63)
#   qT  [128=d-in-pair, HP, S]     scores rhs (from PE transposes of qn)
#   qn  [128=k-in-tile, S/128, 8*(D+1)]  attnV lhsT; col D of each head slot
#                                  is a ones column -> PSUM row 64 accumulates
#                                  the softmax denominator for free.
#   scores computed transposed (scoresT[k, q] = K @ Q^T); causal masking via
#   column-trimmed ranges + one triu multiply on the frontier 128-block.
#
# Schedule: chunk-outer / head-pair-inner attention in chunk order 0,2,1,3
# so each chunk's AllGather is emitted one stream after its normalization
# and consumed one stream later still. attnV for unit t is emitted during
# unit t+1 (software pipeline) so the PE never waits in the
# scores->exp->attnV chain; projection/outproj/exchange groups are doled out
# between attention units to keep the PE busy while the ACT engine (exp)
# catches up. Only the last chunk's exchange is tail-serial; chunk 1's
# output projection and warm matmuls (which also hold the HAM clock at 8/8)
# fill that window.

import sys

for _p in ("/opt/trn_rl_repo",):
    if _p not in sys.path:
        sys.path.append(_p)

import numpy as np
import ml_dtypes

BF16 = ml_dtypes.bfloat16

B, S, E, H = 4, 2048, 1024, 16
D = E // H
NCORES = 8
NH = H // 2          # local heads per core
HP = NH // 2         # local head pairs

_CACHE = {}


def _build_program(S, E, H, n_cores=NCORES):
    import concourse.bass as bass
    import concourse.mybir as mybir
    import concourse.tile as tile
    from concourse import bacc
    from contextlib import ExitStack

    P = 128
    D = E // H
    NH = H // 2
    HP = NH // 2
    assert D == 64 and S % 512 == 0 and E % P == 0
    S_t = S // P          # seq tiles (16)
    E_t = E // P          # embed tiles (8)
    EH = NH * D           # own hidden dims (512)
    CH = 512              # q chunk
    spc = CH // P         # subtiles per chunk (4)
    n_ch = S // CH        # chunks (4)
    Lq = S // 2           # output rows per core
    f32 = mybir.dt.float32
    bf16 = mybir.dt.bfloat16
    Exp = mybir.ActivationFunctionType.Exp
    Ident = mybir.ActivationFunctionType.Identity
    scale = 1.0 / float(np.sqrt(E))
    groups = [[2 * i, 2 * i + 1] for i in range(n_cores // 2)]

    nc = bacc.Bacc(
        "TRN2", target_bir_lowering=False, debug=False, num_devices=n_cores
    )

    xT_d = nc.dram_tensor("xT", [E, S], bf16, kind="ExternalInput").ap()
    wqT_d = nc.dram_tensor("wqT", [E, EH], bf16, kind="ExternalInput").ap()
    wkT_d = nc.dram_tensor("wkT", [E, EH], bf16, kind="ExternalInput").ap()
    # full hidden rows x my 512 output columns (e-split output projection)
    woT_d = nc.dram_tensor("woT", [E, EH], bf16, kind="ExternalInput").ap()
    bo_d = nc.dram_tensor("bo", [P, EH // P], f32, kind="ExternalInput").ap()
    consts_d = nc.dram_tensor("consts", [P, 2 * P], bf16,
                              kind="ExternalInput").ap()
    # transposed output: my 512 e-columns for ALL queries
    out_d = nc.dram_tensor("out", [EH, S], bf16, kind="ExternalOutput").ap()

    with tile.TileContext(nc) as tc, ExitStack() as ctx:
        main = ctx.enter_context(tc.tile_pool(name="main", bufs=1))
        expp = ctx.enter_context(tc.tile_pool(name="expp", bufs=3))
        stgp = ctx.enter_context(tc.tile_pool(name="stgp", bufs=2))
        ostp = ctx.enter_context(tc.tile_pool(name="ostp", bufs=3))
        dram = ctx.enter_context(tc.tile_pool(name="dram", bufs=1,
                                              space="DRAM"))

        xTa = main.tile([P, E_t, S], bf16, tag="xTa", name="xTa")
        wqf = main.tile([P, E_t, EH], bf16)
        wkf = main.tile([P, E_t, EH], bf16)
        wo = main.tile([P, E_t, EH], bf16)
        qn = main.tile([P, S_t, NH * (D + 1)], bf16)
        kT = main.tile([P, HP, S], bf16)
        qT = main.tile([P, HP, S], bf16)
        attnT = main.tile([P, HP, S], bf16)
        # all 16 heads' attn (own + peer, head order 0..15), via AllGather
        attnF = main.tile([P, 2 * HP, S], bf16)
        consts = main.tile([P, 2 * P], bf16)
        bo_sb = main.tile([P, EH // P], f32)
        ones128 = main.tile([1, P], bf16)
        wsrc = main.tile([P, 256], bf16)

        # attention chunks as (first q-tile, width in tiles). NOTE: splitting
        # the last chunk into two 256-col chunks was tried and REGRESSED
        # (+53us): the extra per-unit overhead and three back-to-back
        # AllGathers on the serial gpsimd queue cost more than the smaller
        # tail exchange saved.
        CHUNKS = [(0, 4), (4, 4), (8, 4), (12, 4)]
        bxi = [dram.tile([HP * P, w * P], bf16, tag=f"bxi{c}", name=f"bxi{c}")
               for c, (j0, w) in enumerate(CHUNKS)]
        bxo = [dram.tile([2 * HP * P, w * P], bf16, tag=f"bxo{c}",
                         name=f"bxo{c}") for c, (j0, w) in enumerate(CHUNKS)]

        nc.vector.memset(ones128, 1.0)
        nc.vector.memset(wsrc, 0.0)

        ident = consts[:, 0:P]
        tri = consts[:, P:2 * P]
        qn4 = qn.rearrange("p t (h c) -> p t h c", c=D + 1)
        xT_r = xT_d.rearrange("(t p) s -> p t s", p=P)
        wq_r = wqT_d.rearrange("(t p) d -> p t d", p=P)
        wk_r = wkT_d.rearrange("(t p) d -> p t d", p=P)
        wo_r = woT_d.rearrange("(t p) e -> p t e", p=P)

        # ---- DMA issue order: first-needed first. Per-e-tile configs: one
        # big multi-tile config per tensor was tried and REGRESSED (+58us) —
        # small configs spread across more parallel DMA queues.
        for e in range(E_t):
            nc.sync.dma_start(out=wqf[:, e, :], in_=wq_r[:, e, :])
        for e in range(E_t):
            nc.sync.dma_start(out=xTa[:, e, 0:CH], in_=xT_r[:, e, 0:CH])
        for e in range(E_t):
            nc.sync.dma_start(out=wkf[:, e, :], in_=wk_r[:, e, :])
        nc.sync.dma_start(out=consts, in_=consts_d)
        for sc in range(1, n_ch):
            for e in range(E_t):
                nc.sync.dma_start(
                    out=xTa[:, e, sc * CH:(sc + 1) * CH],
                    in_=xT_r[:, e, sc * CH:(sc + 1) * CH],
                )
        nc.sync.dma_start(out=bo_sb, in_=bo_d)
        for e in range(E_t):
            nc.sync.dma_start(out=wo[:, e, :], in_=wo_r[:, e, :])

        # ---- PE warmup: keep HAM at 8/8 while the initial DMAs stream ----
        with tc.tile_pool(name="wps", bufs=1, space="PSUM") as wpsp:
            wdst = wpsp.tile([P, 256], f32)
            for _ in range(80):
                nc.tensor.matmul(wdst, wsrc[:, 0:P], wsrc, start=True,
                                 stop=True)

        pproj = ctx.enter_context(
            tc.tile_pool(name="pproj", bufs=2, space="PSUM"))
        psc = ctx.enter_context(
            tc.tile_pool(name="psc", bufs=2, space="PSUM"))
        pav = ctx.enter_context(
            tc.tile_pool(name="pav", bufs=1, space="PSUM"))

        # ---- task groups ----
        def qn_group(st):
            ps = pproj.tile([P, CH], f32, tag="ps", name="ps")
            for e in range(E_t):
                nc.tensor.matmul(
                    ps,
                    xTa[:, e, st * P:(st + 1) * P],
                    wqf[:, e, :],
                    start=(e == 0),
                    stop=(e == E_t - 1),
                )
            nc.vector.tensor_copy(
                out=qn4[:, st, :, 0:D],
                in_=ps.rearrange("p (h c) -> p h c", c=D),
            )
            nc.gpsimd.memset(qn4[:, st, :, D:D + 1], 1.0)

        def k_group(hp, sc):
            ps = pproj.tile([P, CH], f32, tag="ps", name="ps")
            for e in range(E_t):
                nc.tensor.matmul(
                    ps,
                    wkf[:, e, hp * P:(hp + 1) * P],
                    xTa[:, e, sc * CH:(sc + 1) * CH],
                    start=(e == 0),
                    stop=(e == E_t - 1),
                )
            nc.vector.tensor_copy(out=kT[:, hp, sc * CH:(sc + 1) * CH],
                                  in_=ps)

        def t_group(hp, qc):
            # transpose qn d-blocks of 4 seq tiles into qT for one head pair.
            # (A single strided-lhsT transpose covering both heads was tried;
            # walrus rejects the strided transpose weight AP.)
            for st in range(4 * qc, 4 * qc + 4):
                ps = pproj.tile([P, CH], f32, tag="ps", name="ps")
                pt = ps.bitcast(bf16)
                nc.tensor.transpose(
                    pt[0:D, 0:P], qn4[:, st, 2 * hp, 0:D], ident)
                nc.tensor.transpose(
                    pt[D:P, 0:P], qn4[:, st, 2 * hp + 1, 0:D], ident)
                nc.vector.tensor_copy(
                    out=qT[:, hp, st * P:(st + 1) * P], in_=pt[:, 0:P])

        def x_group(cid):
            # exchange chunk cid of attnT with the pair peer via AllGather;
            # both contributions land in attnF in fixed head order 0..15, so
            # the program stays uniform. Post-CC DMAs ride the gpsimd queue.
            j0, w = CHUNKS[cid]
            base, CHc = j0 * P, w * P
            nc.sync.dma_start(
                out=bxi[cid].rearrange("(hp p) q -> p hp q", p=P),
                in_=attnT[:, :, base:base + CHc])
            nc.gpsimd.collective_compute(
                "AllGather", mybir.AluOpType.bypass,
                replica_groups=groups,
                ins=[bxi[cid].opt()], outs=[bxo[cid].opt()],
            )
            nc.gpsimd.dma_start(
                out=attnF[:, :, base:base + CHc],
                in_=bxo[cid].rearrange("(s p) q -> p s q", p=P))

        def o_group(cid, et):
            # output projection for chunk cid, e-tile et of my 512 columns,
            # over all 16 heads (attnF), bias added on the ACT-engine evict
            j0, w = CHUNKS[cid]
            base, CHc = j0 * P, w * P
            ps = pproj.tile([P, CH], f32, tag="ps", name="ps")
            for cp in range(2 * HP):
                nc.tensor.matmul(
                    ps[:, 0:CHc],
                    wo[:, cp, et * P:(et + 1) * P],
                    attnF[:, cp, base:base + CHc],
                    start=(cp == 0),
                    stop=(cp == 2 * HP - 1),
                )
            ot = ostp.tile([P, CH], bf16, tag="ot")
            nc.scalar.activation(out=ot[:, 0:CHc], in_=ps[:, 0:CHc],
                                 func=Ident, bias=bo_sb[:, et:et + 1])
            nc.sync.dma_start(
                out=out_d[et * P:(et + 1) * P, base:base + CHc],
                in_=ot[:, 0:CHc])

        # ---- prefix: projections needed by the qc0 attention units ----
        for st in range(4):
            qn_group(st)
        for hp in range(HP):
            k_group(hp, 0)
        for hp in range(HP):
            t_group(hp, 0)

        # ---- attention, qc-outer ----
        pending = []

        def flush_pending(use_pav=False):
            # NOTE: a [1,CH] reciprocal + partition_broadcast multiply was
            # tried to avoid the PE broadcast matmul, but DVE rejects
            # zero-step partition APs; the ones-outer-product stays.
            for (php, pcid, stgs) in pending:
                pj0, pw = CHUNKS[pcid]
                pbase, pCHc = pj0 * P, pw * P
                for half in range(2):
                    if use_pav:
                        rb = pav.tile([P, CH], f32,
                                      tag=("pvA" if half == 0 else "pvB"),
                                      name="rb")
                    else:
                        rb = pproj.tile([P, CH], f32, tag="ps", name="rb")
                    nc.tensor.matmul(rb[:, 0:pCHc], ones128[0:1, :],
                                     stgs[half][:, 0:pCHc],
                                     start=True, stop=True)
                    rcp = stgp.tile([P, CH], f32, tag="rbs", bufs=1)
                    nc.vector.reciprocal_approx_fast(out=rcp[:, 0:pCHc],
                                                     in_=rb[:, 0:pCHc])
                    dst = attnT[half * D:(half + 1) * D, php,
                                pbase:pbase + pCHc]
                    nc.vector.tensor_tensor(out=dst, in0=dst,
                                            in1=rcp[half * D:(half + 1) * D,
                                                    0:pCHc],
                                            op=mybir.AluOpType.mult)
            pending.clear()

        # chunk order 0,2,1,3,4: each chunk's AllGather exchange is emitted
        # one stream after its normalization and consumed (by the output
        # projection) later still, hiding the collective latency; only the
        # final 256-col chunk's exchange is tail-serial.
        qorder = [0, 2, 1, 3]
        NE = EH // P  # my output e-tiles (4)
        stream_tasks = {
            0: ([lambda st=st: qn_group(st) for st in range(4, 16)]
                + [lambda hp=hp, sc=sc: k_group(hp, sc)
                   for sc in (1, 2) for hp in range(HP)]
                + [lambda hp=hp: t_group(hp, 2) for hp in range(HP)]),
            2: ([lambda: x_group(0)]
                + [lambda hp=hp: t_group(hp, 1) for hp in range(HP)]),
            1: ([lambda: x_group(2)]
                + [lambda hp=hp: t_group(hp, 3) for hp in range(HP)]
                + [lambda hp=hp: k_group(hp, 3) for hp in range(HP)]
                + [lambda et=et: o_group(0, et) for et in range(NE)]),
            3: ([lambda: x_group(1)]
                + [lambda et=et: o_group(2, et) for et in range(NE)]),
        }

        for cid in qorder:
            j0, w = CHUNKS[cid]
            base, CHc = j0 * P, w * P
            tasks = stream_tasks[cid]
            ti = 0
            t_max = j0 + w - 1
            tstart = 3
            slots = HP * max(1, t_max + 1 - tstart)
            tspace = max(1, slots // max(1, len(tasks)))
            for hp in range(HP):
                hA, hB = 2 * hp, 2 * hp + 1
                pvA = pav.tile([P, CH], f32, tag="pvA")
                pvB = pav.tile([P, CH], f32, tag="pvB")
                # software pipeline: attnV for unit t is emitted during unit
                # t+1, after the next scores+exp have been issued, so the PE
                # never sits in the scores->exp->attnV dependency chain.
                pipe = None

                def attn_v(ex, t, qoff):
                    nc.tensor.matmul(
                        pvA[0:D + 1, qoff:CHc],
                        qn[:, t, hA * (D + 1):(hA + 1) * (D + 1)],
                        ex[:, 0, qoff:CHc],
                        start=(t == 0),
                        stop=(t == t_max),
                    )
                    nc.tensor.matmul(
                        pvB[0:D + 1, qoff:CHc],
                        qn[:, t, hB * (D + 1):(hB + 1) * (D + 1)],
                        ex[:, 1, qoff:CHc],
                        start=(t == 0),
                        stop=(t == t_max),
                    )

                for t in range(t_max + 1):
                    if t == 2:
                        flush_pending()
                    jloc = max(0, t - j0)
                    qoff = jloc * P
                    sc_t = psc.tile([P, 2, CH], f32, tag="sc_t")
                    nc.tensor.matmul(
                        sc_t[:, 0, qoff:CHc],
                        kT[0:D, hp, t * P:(t + 1) * P],
                        qT[0:D, hp, base + qoff:base + CHc],
                        start=True, stop=True,
                    )
                    nc.tensor.matmul(
                        sc_t[:, 1, qoff:CHc],
                        kT[D:P, hp, t * P:(t + 1) * P],
                        qT[D:P, hp, base + qoff:base + CHc],
                        start=True, stop=True,
                    )
                    ex = expp.tile([P, 2, CH], bf16)
                    nc.scalar.activation(
                        out=ex[:, :, qoff:CHc],
                        in_=sc_t[:, :, qoff:CHc],
                        func=Exp,
                        scale=scale,
                    )
                    if t >= j0:
                        # frontier 128-block: triu multiply (vector, NOT
                        # gpsimd: the collectives block the gpsimd queue)
                        for h2 in range(2):
                            nc.vector.tensor_mul(
                                out=ex[:, h2, qoff:qoff + P],
                                in0=ex[:, h2, qoff:qoff + P],
                                in1=tri,
                            )
                    if pipe is not None:
                        attn_v(*pipe)
                    pipe = (ex, t, qoff)
                    if (t >= tstart and (t - tstart) % tspace == 0
                            and ti < len(tasks)):
                        tasks[ti]()
                        ti += 1
                attn_v(*pipe)
                # evict unnormalized attn + rowsum row; queue normalization
                stgs = []
                for pv, half in ((pvA, 0), (pvB, 1)):
                    stg = stgp.tile([1, CH], bf16, tag="stg", bufs=2)
                    nc.vector.tensor_copy(out=stg[:, 0:CHc],
                                          in_=pv[D:D + 1, 0:CHc])
                    nc.vector.tensor_copy(
                        out=attnT[half * D:(half + 1) * D, hp,
                                  base:base + CHc],
                        in_=pv[0:D, 0:CHc],
                    )
                    stgs.append(stg)
                pending.append((hp, cid, stgs))
            while ti < len(tasks):
                tasks[ti]()
                ti += 1

        # flush the last pending normalization into spare pav banks, then
        # tail: final 256-col exchange; chunk 3's output projection and warm
        # matmuls fill the PE while the last AllGather is in flight
        wdst2 = psc.tile([P, 2, CH], f32, tag="sc_t")
        for _ in range(4):
            nc.tensor.matmul(wdst2[:, 0, 0:256], wsrc[:, 0:P], wsrc,
                             start=True, stop=True)
        flush_pending(use_pav=True)
        x_group(3)
        for et in range(NE):
            o_group(1, et)
        wdst3 = psc.tile([P, 2, CH], f32, tag="sc_t", name="wdst3")
        for _ in range(120):
            nc.tensor.matmul(wdst3[:, 0, 0:256], wsrc[:, 0:P], wsrc,
                             start=True, stop=True)
        for et in range(NE):
            o_group(3, et)

    nc.finalize()
    return nc


def _prep_inputs(x, Wk, Wq, Wo, bo, n_cores=NCORES):
    """Per-core input maps: batch = c//2, head half = c%2 (all bf16).

    wq/wk columns select the core's 8 heads; wo columns select the core's
    512 OUTPUT dims (e-split outproj over all 16 heads via the exchange).
    """
    b, s, e = x.shape
    P = 128
    EH = e // 2
    wqT = np.ascontiguousarray(Wq.T).astype(BF16)
    wkT = np.ascontiguousarray(Wk.T).astype(BF16)
    woT = np.ascontiguousarray(Wo.T).astype(BF16)
    consts = np.concatenate(
        [np.eye(P, dtype=np.float32),
         np.triu(np.ones((P, P), dtype=np.float32))], axis=1).astype(BF16)
    in_maps = []
    for c in range(n_cores):
        bi, hh = c // 2, c % 2
        xT = np.ascontiguousarray(x[bi].T).astype(BF16)
        bo_col = np.ascontiguousarray(
            bo[hh * EH:(hh + 1) * EH].reshape(EH // P, P).T
        ).astype(np.float32)
        in_maps.append({
            "xT": xT,
            "wqT": np.ascontiguousarray(wqT[:, hh * EH:(hh + 1) * EH]),
            "wkT": np.ascontiguousarray(wkT[:, hh * EH:(hh + 1) * EH]),
            "woT": np.ascontiguousarray(woT[:, hh * EH:(hh + 1) * EH]),
            "bo": bo_col,
            "consts": consts,
        })
    return in_maps


def kernel(x, Wk, Wq, Wv, Wo, bo):
    from concourse import bass_utils

    x = np.asarray(x, dtype=np.float32)
    Wk = np.asarray(Wk, dtype=np.float32)
    Wq = np.asarray(Wq, dtype=np.float32)
    Wo = np.asarray(Wo, dtype=np.float32)
    bo = np.asarray(bo, dtype=np.float32)
    b, s, e = x.shape
    key = (s, e, H)
    if key not in _CACHE:
        _CACHE[key] = _build_program(s, e, H)
    nc = _CACHE[key]
    in_maps = _prep_inputs(x, Wk, Wq, Wo, bo)
    res = bass_utils.run_bass_kernel_spmd(nc, in_maps, list(range(NCORES)))
    out = np.empty((b, s, e), dtype=np.float32)
    EH = e // 2
    for c in range(NCORES):
        bi, hh = c // 2, c % 2
        oc = np.asarray(res.results[c]["out"], dtype=np.float32)  # [EH, S]
        out[bi, :, hh * EH:(hh + 1) * EH] = oc.T
    return out


if __name__ == "__main__":
    nc = _build_program(S, E, H)
    print("built ok")


# revision 52
# speedup vs baseline: 1.0138x; 1.0138x over previous
# Multi-head masked attention (V = Q source quirk; Wv unused) on 8 TRN2 NeuronCores.
#
# Sharding: 8 cores = 4 batches x 2 head-halves (tensor parallel). Core c
# handles batch b = c//2 and heads hh*8..hh*8+7 (hh = c%2), for ALL queries.
# Each core projects K^T and Q-natural (= V) for its own 8 heads only (no
# duplicated projection work across the pair, unlike a query split), derives
# Q^T from Q-natural via PE transposes (4x cheaper than re-projecting), and
# runs causal attention for its heads over all 2048 queries. The pair then
# exchanges normalized attention outputs per query-chunk with pairwise
# AllGather collectives (chosen over ReduceScatter: ~16us vs ~45us per op on
# this NRT), after which each core runs the output projection over all 16
# heads for ITS 512 output columns (Wo column-half + bias live in the
# per-core input data) and writes out^T[my_e, all_q] directly. The program
# is fully SPMD-uniform: head/e-column assignment is carried by input data,
# and both pair contributions come back from the AllGather in fixed device
# order, so no rank-dependent addressing exists anywhere.
#
# Layouts (per core, bf16 matmul operands, fp32 PSUM accumulation):
#   kT  [128=d-in-pair, HP=4, S]   scores lhsT  (head even: partitions 0-63)
#   qT  [128=d-in-pair, HP, S]     scores rhs (from PE transposes of qn)
#   qn  [128=k-in-tile, S/128, 8*(D+1)]  attnV lhsT; col D of each head slot
#                                  is a ones column -> PSUM row 64 accumulates
#                                  the softmax denominator for free.
#   scores computed transposed (scoresT[k, q] = K @ Q^T); causal masking via
#   column-trimmed ranges + one triu multiply on the frontier 128-block.
#
# Schedule: chunk-outer / head-pair-inner attention in chunk order 0,2,1,3
# so each chunk's AllGather is emitted one stream after its normalization
# and consumed one stream later still. attnV for unit t is emitted during
# unit t+1 (software pipeline) so the PE never waits in the
# scores->exp->attnV chain; projection/outproj/exchange groups are doled out
# between attention units to keep the PE busy while the ACT engine (exp)
# catches up. Only the last chunk's exchange is tail-serial; chunk 1's
# output projection and warm matmuls (which also hold the HAM clock at 8/8)
# fill that window.

import sys

for _p in ("/opt/trn_rl_repo",):
    if _p not in sys.path:
        sys.path.append(_p)

import numpy as np
import ml_dtypes

BF16 = ml_dtypes.bfloat16

B, S, E, H = 4, 2048, 1024, 16
D = E // H
NCORES = 8
NH = H // 2          # local heads per core
HP = NH // 2         # local head pairs

_CACHE = {}


def _build_program(S, E, H, n_cores=NCORES):
    import concourse.bass as bass
    import concourse.mybir as mybir
    import concourse.tile as tile
    from concourse import bacc
    from contextlib import ExitStack

    P = 128
    D = E // H
    NH = H // 2
    HP = NH // 2
    assert D == 64 and S % 512 == 0 and E % P == 0
    S_t = S // P          # seq tiles (16)
    E_t = E // P          # embed tiles (8)
    EH = NH * D           # own hidden dims (512)
    CH = 512              # q chunk
    spc = CH // P         # subtiles per chunk (4)
    n_ch = S // CH        # chunks (4)
    Lq = S // 2           # output rows per core
    f32 = mybir.dt.float32
    bf16 = mybir.dt.bfloat16
    Exp = mybir.ActivationFunctionType.Exp
    Ident = mybir.ActivationFunctionType.Identity
    scale = 1.0 / float(np.sqrt(E))
    groups = [[2 * i, 2 * i + 1] for i in range(n_cores // 2)]

    nc = bacc.Bacc(
        "TRN2", target_bir_lowering=False, debug=False, num_devices=n_cores
    )

    xT_d = nc.dram_tensor("xT", [E, S], bf16, kind="ExternalInput").ap()
    wqT_d = nc.dram_tensor("wqT", [E, EH], bf16, kind="ExternalInput").ap()
    wkT_d = nc.dram_tensor("wkT", [E, EH], bf16, kind="ExternalInput").ap()
    # full hidden rows x my 512 output columns (e-split output projection)
    woT_d = nc.dram_tensor("woT", [E, EH], bf16, kind="ExternalInput").ap()
    bo_d = nc.dram_tensor("bo", [P, EH // P], f32, kind="ExternalInput").ap()
    consts_d = nc.dram_tensor("consts", [P, 2 * P], bf16,
                              kind="ExternalInput").ap()
    # transposed output: my 512 e-columns for ALL queries
    out_d = nc.dram_tensor("out", [EH, S], bf16, kind="ExternalOutput").ap()

    with tile.TileContext(nc) as tc, ExitStack() as ctx:
        main = ctx.enter_context(tc.tile_pool(name="main", bufs=1))
        expp = ctx.enter_context(tc.tile_pool(name="expp", bufs=3))
        stgp = ctx.enter_context(tc.tile_pool(name="stgp", bufs=2))
        ostp = ctx.enter_context(tc.tile_pool(name="ostp", bufs=3))
        dram = ctx.enter_context(tc.tile_pool(name="dram", bufs=1,
                                              space="DRAM"))

        xTa = main.tile([P, E_t, S], bf16, tag="xTa", name="xTa")
        wqf = main.tile([P, E_t, EH], bf16)
        wkf = main.tile([P, E_t, EH], bf16)
        wo = main.tile([P, E_t, EH], bf16)
        qn = main.tile([P, S_t, NH * (D + 1)], bf16)
        kT = main.tile([P, HP, S], bf16)
        qT = main.tile([P, HP, S], bf16)
        attnT = main.tile([P, HP, S], bf16)
        # all 16 heads' attn (own + peer, head order 0..15), via AllGather
        attnF = main.tile([P, 2 * HP, S], bf16)
        consts = main.tile([P, 2 * P], bf16)
        bo_sb = main.tile([P, EH // P], f32)
        ones128 = main.tile([1, P], bf16)
        wsrc = main.tile([P, 256], bf16)

        # attention chunks as (first q-tile, width in tiles). NOTE: splitting
        # the last chunk into two 256-col chunks was tried and REGRESSED
        # (+53us): the extra per-unit overhead and three back-to-back
        # AllGathers on the serial gpsimd queue cost more than the smaller
        # tail exchange saved.
        CHUNKS = [(0, 4), (4, 4), (8, 4), (12, 4)]
        bxi = [dram.tile([HP * P, w * P], bf16, tag=f"bxi{c}", name=f"bxi{c}")
               for c, (j0, w) in enumerate(CHUNKS)]
        bxo = [dram.tile([2 * HP * P, w * P], bf16, tag=f"bxo{c}",
                         name=f"bxo{c}") for c, (j0, w) in enumerate(CHUNKS)]
        # the last chunk's exchange is split by head-pair halves: 3a (hp0-1)
        # overlaps the tail of its own attention stream, only 3b (hp2-3,
        # 256KB) is tail-serial
        bxi3a = dram.tile([2 * P, CH], bf16, tag="bxi3a", name="bxi3a")
        bxo3a = dram.tile([4 * P, CH], bf16, tag="bxo3a", name="bxo3a")
        bxi3b = dram.tile([2 * P, CH], bf16, tag="bxi3b", name="bxi3b")
        bxo3b = dram.tile([4 * P, CH], bf16, tag="bxo3b", name="bxo3b")

        nc.vector.memset(ones128, 1.0)
        nc.vector.memset(wsrc, 0.0)

        ident = consts[:, 0:P]
        tri = consts[:, P:2 * P]
        qn4 = qn.rearrange("p t (h c) -> p t h c", c=D + 1)
        xT_r = xT_d.rearrange("(t p) s -> p t s", p=P)
        wq_r = wqT_d.rearrange("(t p) d -> p t d", p=P)
        wk_r = wkT_d.rearrange("(t p) d -> p t d", p=P)
        wo_r = woT_d.rearrange("(t p) e -> p t e", p=P)

        # ---- DMA issue order: first-needed first. Per-e-tile configs: one
        # big multi-tile config per tensor was tried and REGRESSED (+58us) —
        # small configs spread across more parallel DMA queues.
        for e in range(E_t):
            nc.sync.dma_start(out=wqf[:, e, :], in_=wq_r[:, e, :])
        for e in range(E_t):
            nc.sync.dma_start(out=xTa[:, e, 0:CH], in_=xT_r[:, e, 0:CH])
        for e in range(E_t):
            nc.sync.dma_start(out=wkf[:, e, :], in_=wk_r[:, e, :])
        nc.sync.dma_start(out=consts, in_=consts_d)
        for sc in range(1, n_ch):
            for e in range(E_t):
                nc.sync.dma_start(
                    out=xTa[:, e, sc * CH:(sc + 1) * CH],
                    in_=xT_r[:, e, sc * CH:(sc + 1) * CH],
                )
        nc.sync.dma_start(out=bo_sb, in_=bo_d)
        for e in range(E_t):
            nc.sync.dma_start(out=wo[:, e, :], in_=wo_r[:, e, :])

        # ---- PE warmup: keep HAM at 8/8 while the initial DMAs stream ----
        with tc.tile_pool(name="wps", bufs=1, space="PSUM") as wpsp:
            wdst = wpsp.tile([P, 256], f32)
            for _ in range(68):
                nc.tensor.matmul(wdst, wsrc[:, 0:P], wsrc, start=True,
                                 stop=True)

        pproj = ctx.enter_context(
            tc.tile_pool(name="pproj", bufs=2, space="PSUM"))
        psc = ctx.enter_context(
            tc.tile_pool(name="psc", bufs=2, space="PSUM"))
        pav = ctx.enter_context(
            tc.tile_pool(name="pav", bufs=1, space="PSUM"))

        # ---- task groups ----
        def qn_group(st):
            ps = pproj.tile([P, CH], f32, tag="ps", name="ps")
            for e in range(E_t):
                nc.tensor.matmul(
                    ps,
                    xTa[:, e, st * P:(st + 1) * P],
                    wqf[:, e, :],
                    start=(e == 0),
                    stop=(e == E_t - 1),
                )
            nc.vector.tensor_copy(
                out=qn4[:, st, :, 0:D],
                in_=ps.rearrange("p (h c) -> p h c", c=D),
            )
            nc.gpsimd.memset(qn4[:, st, :, D:D + 1], 1.0)

        def k_group(hp, sc):
            ps = pproj.tile([P, CH], f32, tag="ps", name="ps")
            for e in range(E_t):
                nc.tensor.matmul(
                    ps,
                    wkf[:, e, hp * P:(hp + 1) * P],
                    xTa[:, e, sc * CH:(sc + 1) * CH],
                    start=(e == 0),
                    stop=(e == E_t - 1),
                )
            nc.vector.tensor_copy(out=kT[:, hp, sc * CH:(sc + 1) * CH],
                                  in_=ps)

        def t_group(hp, qc):
            # transpose qn d-blocks of 4 seq tiles into qT for one head pair.
            # (A single strided-lhsT transpose covering both heads was tried;
            # walrus rejects the strided transpose weight AP.)
            for st in range(4 * qc, 4 * qc + 4):
                ps = pproj.tile([P, CH], f32, tag="ps", name="ps")
                pt = ps.bitcast(bf16)
                nc.tensor.transpose(
                    pt[0:D, 0:P], qn4[:, st, 2 * hp, 0:D], ident)
                nc.tensor.transpose(
                    pt[D:P, 0:P], qn4[:, st, 2 * hp + 1, 0:D], ident)
                nc.vector.tensor_copy(
                    out=qT[:, hp, st * P:(st + 1) * P], in_=pt[:, 0:P])

        def x_group(cid):
            # exchange chunk cid of attnT with the pair peer via AllGather;
            # both contributions land in attnF in fixed head order 0..15, so
            # the program stays uniform. Post-CC DMAs ride the gpsimd queue.
            j0, w = CHUNKS[cid]
            base, CHc = j0 * P, w * P
            # per-head-pair bounce writes: hp0-2's rows stream out while the
            # last head pair's normalization finishes
            bxi_r = bxi[cid].rearrange("(hp p) q -> p hp q", p=P)
            for hp in range(HP):
                nc.sync.dma_start(
                    out=bxi_r[:, hp, :],
                    in_=attnT[:, hp, base:base + CHc])
            nc.gpsimd.collective_compute(
                "AllGather", mybir.AluOpType.bypass,
                replica_groups=groups,
                ins=[bxi[cid].opt()], outs=[bxo[cid].opt()],
            )
            nc.gpsimd.dma_start(
                out=attnF[:, :, base:base + CHc],
                in_=bxo[cid].rearrange("(s p) q -> p s q", p=P))

        def x3_half(hp0, bi, bo):
            # exchange head pairs hp0, hp0+1 of the LAST chunk. AG output
            # blocks: [dev0-hpA, dev0-hpB, dev1-hpA, dev1-hpB] -> attnF head
            # slots (hp0, hp0+1) and (4+hp0, 5+hp0)
            j0, w = CHUNKS[-1]
            base, CHc = j0 * P, w * P
            bi_r = bi.rearrange("(hp p) q -> p hp q", p=P)
            for k in range(2):
                nc.sync.dma_start(
                    out=bi_r[:, k, :],
                    in_=attnT[:, hp0 + k, base:base + CHc])
            nc.gpsimd.collective_compute(
                "AllGather", mybir.AluOpType.bypass,
                replica_groups=groups,
                ins=[bi.opt()], outs=[bo.opt()],
            )
            bo_r = bo.rearrange("(s p) q -> p s q", p=P)
            nc.gpsimd.dma_start(
                out=attnF[:, hp0:hp0 + 2, base:base + CHc],
                in_=bo_r[:, 0:2, :])
            nc.gpsimd.dma_start(
                out=attnF[:, HP + hp0:HP + hp0 + 2, base:base + CHc],
                in_=bo_r[:, 2:4, :])

        def o_group(cid, et):
            # output projection for chunk cid, e-tile et of my 512 columns,
            # over all 16 heads (attnF), bias added on the ACT-engine evict
            j0, w = CHUNKS[cid]
            base, CHc = j0 * P, w * P
            ps = pproj.tile([P, CH], f32, tag="ps", name="ps")
            for cp in range(2 * HP):
                nc.tensor.matmul(
                    ps[:, 0:CHc],
                    wo[:, cp, et * P:(et + 1) * P],
                    attnF[:, cp, base:base + CHc],
                    start=(cp == 0),
                    stop=(cp == 2 * HP - 1),
                )
            ot = ostp.tile([P, CH], bf16, tag="ot")
            nc.scalar.activation(out=ot[:, 0:CHc], in_=ps[:, 0:CHc],
                                 func=Ident, bias=bo_sb[:, et:et + 1])
            nc.sync.dma_start(
                out=out_d[et * P:(et + 1) * P, base:base + CHc],
                in_=ot[:, 0:CHc])

        # ---- prefix: projections needed by the qc0 attention units ----
        for st in range(4):
            qn_group(st)
        for hp in range(HP):
            k_group(hp, 0)
        for hp in range(HP):
            t_group(hp, 0)

        # ---- attention, qc-outer ----
        pending = []

        def flush_pending(use_pav=False):
            # NOTE: a [1,CH] reciprocal + partition_broadcast multiply was
            # tried to avoid the PE broadcast matmul, but DVE rejects
            # zero-step partition APs; the ones-outer-product stays.
            for (php, pcid, stgs) in pending:
                pj0, pw = CHUNKS[pcid]
                pbase, pCHc = pj0 * P, pw * P
                for half in range(2):
                    if use_pav:
                        rb = pav.tile([P, CH], f32,
                                      tag=("pvA" if half == 0 else "pvB"),
                                      name="rb")
                    else:
                        rb = pproj.tile([P, CH], f32, tag="ps", name="rb")
                    nc.tensor.matmul(rb[:, 0:pCHc], ones128[0:1, :],
                                     stgs[half][:, 0:pCHc],
                                     start=True, stop=True)
                    rcp = stgp.tile([P, CH], f32, tag="rbs", bufs=1)
                    nc.vector.reciprocal_approx_fast(out=rcp[:, 0:pCHc],
                                                     in_=rb[:, 0:pCHc])
                    dst = attnT[half * D:(half + 1) * D, php,
                                pbase:pbase + pCHc]
                    nc.vector.tensor_tensor(out=dst, in0=dst,
                                            in1=rcp[half * D:(half + 1) * D,
                                                    0:pCHc],
                                            op=mybir.AluOpType.mult)
            pending.clear()

        # chunk order 0,2,1,3,4: each chunk's AllGather exchange is emitted
        # one stream after its normalization and consumed (by the output
        # projection) later still, hiding the collective latency; only the
        # final 256-col chunk's exchange is tail-serial.
        qorder = [0, 2, 1, 3]
        NE = EH // P  # my output e-tiles (4)
        stream_tasks = {
            0: ([lambda st=st: qn_group(st) for st in range(4, 16)]
                + [lambda hp=hp, sc=sc: k_group(hp, sc)
                   for sc in (1, 2) for hp in range(HP)]
                + [lambda hp=hp: t_group(hp, 2) for hp in range(HP)]),
            2: ([lambda: x_group(0)]
                + [lambda hp=hp: t_group(hp, 1) for hp in range(HP)]),
            1: ([lambda: x_group(2)]
                + [lambda hp=hp: t_group(hp, 3) for hp in range(HP)]
                + [lambda hp=hp: k_group(hp, 3) for hp in range(HP)]
                + [lambda et=et: o_group(0, et) for et in range(NE)]),
            3: ([lambda: x_group(1)]
                + [lambda et=et: o_group(2, et) for et in range(NE)]),
        }

        for cid in qorder:
            j0, w = CHUNKS[cid]
            base, CHc = j0 * P, w * P
            tasks = stream_tasks[cid]
            ti = 0
            t_max = j0 + w - 1
            # stream 0 carries 24 projection tasks over only 16 short units:
            # dole several per unit (instead of one, which bunched 20 of
            # them into a solid post-stream block with poorer pacing)
            tstart = 2 if cid == 0 else 3
            slots = HP * max(1, t_max + 1 - tstart)
            tspace = max(1, slots // max(1, len(tasks)))
            tper = -(-len(tasks) // slots)  # ceil: tasks per eligible slot
            for hp in range(HP):
                hA, hB = 2 * hp, 2 * hp + 1
                pvA = pav.tile([P, CH], f32, tag="pvA")
                pvB = pav.tile([P, CH], f32, tag="pvB")
                # software pipeline: attnV for unit t is emitted during unit
                # t+1, after the next scores+exp have been issued, so the PE
                # never sits in the scores->exp->attnV dependency chain.
                pipe = None

                def attn_v(ex, t, qoff):
                    nc.tensor.matmul(
                        pvA[0:D + 1, qoff:CHc],
                        qn[:, t, hA * (D + 1):(hA + 1) * (D + 1)],
                        ex[:, 0, qoff:CHc],
                        start=(t == 0),
                        stop=(t == t_max),
                    )
                    nc.tensor.matmul(
                        pvB[0:D + 1, qoff:CHc],
                        qn[:, t, hB * (D + 1):(hB + 1) * (D + 1)],
                        ex[:, 1, qoff:CHc],
                        start=(t == 0),
                        stop=(t == t_max),
                    )

                for t in range(t_max + 1):
                    if t == 2:
                        flush_pending()
                    jloc = max(0, t - j0)
                    qoff = jloc * P
                    sc_t = psc.tile([P, 2, CH], f32, tag="sc_t")
                    nc.tensor.matmul(
                        sc_t[:, 0, qoff:CHc],
                        kT[0:D, hp, t * P:(t + 1) * P],
                        qT[0:D, hp, base + qoff:base + CHc],
                        start=True, stop=True,
                    )
                    nc.tensor.matmul(
                        sc_t[:, 1, qoff:CHc],
                        kT[D:P, hp, t * P:(t + 1) * P],
                        qT[D:P, hp, base + qoff:base + CHc],
                        start=True, stop=True,
                    )
                    ex = expp.tile([P, 2, CH], bf16)
                    nc.scalar.activation(
                        out=ex[:, :, qoff:CHc],
                        in_=sc_t[:, :, qoff:CHc],
                        func=Exp,
                        scale=scale,
                    )
                    if t >= j0:
                        # frontier 128-block: triu multiply (vector, NOT
                        # gpsimd: the collectives block the gpsimd queue)
                        for h2 in range(2):
                            nc.vector.tensor_mul(
                                out=ex[:, h2, qoff:qoff + P],
                                in0=ex[:, h2, qoff:qoff + P],
                                in1=tri,
                            )
                    if pipe is not None:
                        attn_v(*pipe)
                    pipe = (ex, t, qoff)
                    if (t >= tstart and (t - tstart) % tspace == 0
                            and ti < len(tasks)):
                        for _ in range(tper):
                            if ti < len(tasks):
                                tasks[ti]()
                                ti += 1
                    if cid == 3 and hp == 2 and t == 3:
                        # hp0-1 of the last chunk are normalized by now
                        x3_half(0, bxi3a, bxo3a)
                attn_v(*pipe)
                # evict unnormalized attn + rowsum row; queue normalization
                stgs = []
                for pv, half in ((pvA, 0), (pvB, 1)):
                    stg = stgp.tile([1, CH], bf16, tag="stg", bufs=2)
                    nc.vector.tensor_copy(out=stg[:, 0:CHc],
                                          in_=pv[D:D + 1, 0:CHc])
                    nc.vector.tensor_copy(
                        out=attnT[half * D:(half + 1) * D, hp,
                                  base:base + CHc],
                        in_=pv[0:D, 0:CHc],
                    )
                    stgs.append(stg)
                pending.append((hp, cid, stgs))
            while ti < len(tasks):
                tasks[ti]()
                ti += 1

        # flush the last pending normalization into spare pav banks, then
        # tail: final 256-col exchange; chunk 3's output projection and warm
        # matmuls fill the PE while the last AllGather is in flight
        wdst2 = psc.tile([P, 2, CH], f32, tag="sc_t")
        for _ in range(4):
            nc.tensor.matmul(wdst2[:, 0, 0:256], wsrc[:, 0:P], wsrc,
                             start=True, stop=True)
        flush_pending(use_pav=True)
        x3_half(2, bxi3b, bxo3b)
        for et in range(NE):
            o_group(1, et)
        # 60 warm matmuls: enough to hold the clock at 8/8 through most of
        # the tail AllGather; more (120 tried) overshoots the collective's
        # completion and delays o_group(3) behind the in-order PE queue.
        wdst3 = psc.tile([P, 2, CH], f32, tag="sc_t", name="wdst3")
        for _ in range(60):
            nc.tensor.matmul(wdst3[:, 0, 0:256], wsrc[:, 0:P], wsrc,
                             start=True, stop=True)
        for et in range(NE):
            o_group(3, et)

    nc.finalize()
    return nc


def _prep_inputs(x, Wk, Wq, Wo, bo, n_cores=NCORES):
    """Per-core input maps: batch = c//2, head half = c%2 (all bf16).

    wq/wk columns select the core's 8 heads; wo columns select the core's
    512 OUTPUT dims (e-split outproj over all 16 heads via the exchange).
    """
    b, s, e = x.shape
    P = 128
    EH = e // 2
    wqT = np.ascontiguousarray(Wq.T).astype(BF16)
    wkT = np.ascontiguousarray(Wk.T).astype(BF16)
    woT = np.ascontiguousarray(Wo.T).astype(BF16)
    consts = np.concatenate(
        [np.eye(P, dtype=np.float32),
         np.triu(np.ones((P, P), dtype=np.float32))], axis=1).astype(BF16)
    in_maps = []
    for c in range(n_cores):
        bi, hh = c // 2, c % 2
        xT = np.ascontiguousarray(x[bi].T).astype(BF16)
        bo_col = np.ascontiguousarray(
            bo[hh * EH:(hh + 1) * EH].reshape(EH // P, P).T
        ).astype(np.float32)
        in_maps.append({
            "xT": xT,
            "wqT": np.ascontiguousarray(wqT[:, hh * EH:(hh + 1) * EH]),
            "wkT": np.ascontiguousarray(wkT[:, hh * EH:(hh + 1) * EH]),
            "woT": np.ascontiguousarray(woT[:, hh * EH:(hh + 1) * EH]),
            "bo": bo_col,
            "consts": consts,
        })
    return in_maps


def kernel(x, Wk, Wq, Wv, Wo, bo):
    from concourse import bass_utils

    x = np.asarray(x, dtype=np.float32)
    Wk = np.asarray(Wk, dtype=np.float32)
    Wq = np.asarray(Wq, dtype=np.float32)
    Wo = np.asarray(Wo, dtype=np.float32)
    bo = np.asarray(bo, dtype=np.float32)
    b, s, e = x.shape
    key = (s, e, H)
    if key not in _CACHE:
        _CACHE[key] = _build_program(s, e, H)
    nc = _CACHE[key]
    in_maps = _prep_inputs(x, Wk, Wq, Wo, bo)
    res = bass_utils.run_bass_kernel_spmd(nc, in_maps, list(range(NCORES)))
    out = np.empty((b, s, e), dtype=np.float32)
    EH = e // 2
    for c in range(NCORES):
        bi, hh = c // 2, c % 2
        oc = np.asarray(res.results[c]["out"], dtype=np.float32)  # [EH, S]
        out[bi, :, hh * EH:(hh + 1) * EH] = oc.T
    return out


if __name__ == "__main__":
    nc = _build_program(S, E, H)
    print("built ok")


# revision 56
# speedup vs baseline: 1.2025x; 1.1861x over previous
# Multi-head masked attention (V = Q source quirk; Wv unused) on 8 TRN2 NeuronCores.
#
# Sharding: 8 cores = 4 batches x 2 head-halves (tensor parallel). Core c
# handles batch b = c//2 and heads hh*8..hh*8+7 (hh = c%2), for ALL queries.
# Each core projects K^T and Q-natural (= V) for its own 8 heads only (no
# duplicated projection work across the pair, unlike a query split), derives
# Q^T from Q-natural via PE transposes (4x cheaper than re-projecting), and
# runs causal attention for its heads over all 2048 queries. The pair then
# exchanges normalized attention outputs per query-chunk with pairwise
# AllGather collectives (chosen over ReduceScatter: ~16us vs ~45us per op on
# this NRT), after which each core runs the output projection over all 16
# heads for ITS 512 output columns (Wo column-half + bias live in the
# per-core input data) and writes out^T[my_e, all_q] directly. The program
# is fully SPMD-uniform: head/e-column assignment is carried by input data,
# and both pair contributions come back from the AllGather in fixed device
# order, so no rank-dependent addressing exists anywhere.
#
# Layouts (per core, bf16 matmul operands, fp32 PSUM accumulation):
#   kT  [128=d-in-pair, HP=4, S]   scores lhsT  (head even: partitions 0-63)
#   qT  [128=d-in-pair, HP, S]     scores rhs (from PE transposes of qn)
#   qn  [128=k-in-tile, S/128, 8*(D+1)]  attnV lhsT; col D of each head slot
#                                  is a ones column -> PSUM row 64 accumulates
#                                  the softmax denominator for free.
#   scores computed transposed (scoresT[k, q] = K @ Q^T); causal masking via
#   column-trimmed ranges + one triu multiply on the frontier 128-block.
#
# Schedule: chunk-outer / head-pair-inner attention in chunk order 0,2,1,3
# so each chunk's AllGather is emitted one stream after its normalization
# and consumed one stream later still. attnV for unit t is emitted during
# unit t+1 (software pipeline) so the PE never waits in the
# scores->exp->attnV chain; projection/outproj/exchange groups are doled out
# between attention units to keep the PE busy while the ACT engine (exp)
# catches up. Only the last chunk's exchange is tail-serial; chunk 1's
# output projection and warm matmuls (which also hold the HAM clock at 8/8)
# fill that window.

import sys

for _p in ("/opt/trn_rl_repo",):
    if _p not in sys.path:
        sys.path.append(_p)

import numpy as np
import ml_dtypes

BF16 = ml_dtypes.bfloat16

B, S, E, H = 4, 2048, 1024, 16
D = E // H
NCORES = 8
NH = H // 2          # local heads per core
HP = NH // 2         # local head pairs

_CACHE = {}


def _build_program(S, E, H, n_cores=NCORES):
    import concourse.bass as bass
    import concourse.mybir as mybir
    import concourse.tile as tile
    from concourse import bacc
    from contextlib import ExitStack

    P = 128
    D = E // H
    NH = H // 2
    HP = NH // 2
    assert D == 64 and S % 512 == 0 and E % P == 0
    S_t = S // P          # seq tiles (16)
    E_t = E // P          # embed tiles (8)
    EH = NH * D           # own hidden dims (512)
    CH = 512              # q chunk
    spc = CH // P         # subtiles per chunk (4)
    n_ch = S // CH        # chunks (4)
    Lq = S // 2           # output rows per core
    f32 = mybir.dt.float32
    bf16 = mybir.dt.bfloat16
    Exp = mybir.ActivationFunctionType.Exp
    Ident = mybir.ActivationFunctionType.Identity
    scale = 1.0 / float(np.sqrt(E))
    groups = [[2 * i, 2 * i + 1] for i in range(n_cores // 2)]

    nc = bacc.Bacc(
        "TRN2", target_bir_lowering=False, debug=False, num_devices=n_cores
    )

    xT_d = nc.dram_tensor("xT", [E, S], bf16, kind="ExternalInput").ap()
    wqT_d = nc.dram_tensor("wqT", [E, EH], bf16, kind="ExternalInput").ap()
    wkT_d = nc.dram_tensor("wkT", [E, EH], bf16, kind="ExternalInput").ap()
    # full hidden rows x my 512 output columns (e-split output projection)
    woT_d = nc.dram_tensor("woT", [E, EH], bf16, kind="ExternalInput").ap()
    bo_d = nc.dram_tensor("bo", [P, EH // P], f32, kind="ExternalInput").ap()
    consts_d = nc.dram_tensor("consts", [P, 2 * P], bf16,
                              kind="ExternalInput").ap()
    # transposed output: my 512 e-columns for ALL queries
    out_d = nc.dram_tensor("out", [EH, S], bf16, kind="ExternalOutput").ap()

    with tile.TileContext(nc) as tc, ExitStack() as ctx:
        main = ctx.enter_context(tc.tile_pool(name="main", bufs=1))
        expp = ctx.enter_context(tc.tile_pool(name="expp", bufs=3))
        stgp = ctx.enter_context(tc.tile_pool(name="stgp", bufs=2))
        ostp = ctx.enter_context(tc.tile_pool(name="ostp", bufs=3))
        dram = ctx.enter_context(tc.tile_pool(name="dram", bufs=1,
                                              space="DRAM"))

        xTa = main.tile([P, E_t, S], bf16, tag="xTa", name="xTa")
        wqf = main.tile([P, E_t, EH], bf16)
        wkf = main.tile([P, E_t, EH], bf16)
        wo = main.tile([P, E_t, EH], bf16)
        qn = main.tile([P, S_t, NH * (D + 1)], bf16)
        kT = main.tile([P, HP, S], bf16)
        qT = main.tile([P, HP, S], bf16)
        attnT = main.tile([P, HP, S], bf16)
        # all 16 heads' attn (own + peer, head order 0..15), via AllGather
        attnF = main.tile([P, 2 * HP, S], bf16)
        consts = main.tile([P, 2 * P], bf16)
        bo_sb = main.tile([P, EH // P], f32)
        ones128 = main.tile([1, P], bf16)
        wsrc = main.tile([P, 256], bf16)

        # attention chunks as (first q-tile, width in tiles). NOTE: splitting
        # the last chunk into two 256-col chunks was tried and REGRESSED
        # (+53us): the extra per-unit overhead and three back-to-back
        # AllGathers on the serial gpsimd queue cost more than the smaller
        # tail exchange saved.
        CHUNKS = [(0, 4), (4, 4), (8, 4), (12, 4)]
        bxi = [dram.tile([HP * P, w * P], bf16, tag=f"bxi{c}", name=f"bxi{c}")
               for c, (j0, w) in enumerate(CHUNKS)]
        bxo = [dram.tile([2 * HP * P, w * P], bf16, tag=f"bxo{c}",
                         name=f"bxo{c}") for c, (j0, w) in enumerate(CHUNKS)]


        nc.vector.memset(ones128, 1.0)
        nc.vector.memset(wsrc, 0.0)

        ident = consts[:, 0:P]
        tri = consts[:, P:2 * P]
        qn4 = qn.rearrange("p t (h c) -> p t h c", c=D + 1)
        xT_r = xT_d.rearrange("(t p) s -> p t s", p=P)
        wq_r = wqT_d.rearrange("(t p) d -> p t d", p=P)
        wk_r = wkT_d.rearrange("(t p) d -> p t d", p=P)
        wo_r = woT_d.rearrange("(t p) e -> p t e", p=P)

        # ---- DMA issue order: first-needed first. Per-e-tile configs: one
        # big multi-tile config per tensor was tried and REGRESSED (+58us) —
        # small configs spread across more parallel DMA queues.
        for e in range(E_t):
            nc.sync.dma_start(out=wqf[:, e, :], in_=wq_r[:, e, :])
        for e in range(E_t):
            nc.sync.dma_start(out=xTa[:, e, 0:CH], in_=xT_r[:, e, 0:CH])
        for e in range(E_t):
            nc.sync.dma_start(out=wkf[:, e, :], in_=wk_r[:, e, :])
        nc.sync.dma_start(out=consts, in_=consts_d)
        for sc in range(1, n_ch):
            for e in range(E_t):
                nc.sync.dma_start(
                    out=xTa[:, e, sc * CH:(sc + 1) * CH],
                    in_=xT_r[:, e, sc * CH:(sc + 1) * CH],
                )
        nc.sync.dma_start(out=bo_sb, in_=bo_d)
        for e in range(E_t):
            nc.sync.dma_start(out=wo[:, e, :], in_=wo_r[:, e, :])

        # ---- PE warmup: keep HAM at 8/8 while the initial DMAs stream ----
        with tc.tile_pool(name="wps", bufs=1, space="PSUM") as wpsp:
            wdst = wpsp.tile([P, 256], f32)
            for _ in range(68):
                nc.tensor.matmul(wdst, wsrc[:, 0:P], wsrc, start=True,
                                 stop=True)

        pproj = ctx.enter_context(
            tc.tile_pool(name="pproj", bufs=2, space="PSUM"))
        psc = ctx.enter_context(
            tc.tile_pool(name="psc", bufs=2, space="PSUM"))
        pav = ctx.enter_context(
            tc.tile_pool(name="pav", bufs=1, space="PSUM"))

        # ---- task groups ----
        def qn_group(st):
            ps = pproj.tile([P, CH], f32, tag="ps", name="ps")
            for e in range(E_t):
                nc.tensor.matmul(
                    ps,
                    xTa[:, e, st * P:(st + 1) * P],
                    wqf[:, e, :],
                    start=(e == 0),
                    stop=(e == E_t - 1),
                )
            nc.vector.tensor_copy(
                out=qn4[:, st, :, 0:D],
                in_=ps.rearrange("p (h c) -> p h c", c=D),
            )
            nc.gpsimd.memset(qn4[:, st, :, D:D + 1], 1.0)

        def k_group(hp, sc):
            ps = pproj.tile([P, CH], f32, tag="ps", name="ps")
            for e in range(E_t):
                nc.tensor.matmul(
                    ps,
                    wkf[:, e, hp * P:(hp + 1) * P],
                    xTa[:, e, sc * CH:(sc + 1) * CH],
                    start=(e == 0),
                    stop=(e == E_t - 1),
                )
            nc.vector.tensor_copy(out=kT[:, hp, sc * CH:(sc + 1) * CH],
                                  in_=ps)

        def t_group(hp, qc):
            # transpose qn d-blocks of 4 seq tiles into qT for one head pair.
            # (A single strided-lhsT transpose covering both heads was tried;
            # walrus rejects the strided transpose weight AP.)
            for st in range(4 * qc, 4 * qc + 4):
                ps = pproj.tile([P, CH], f32, tag="ps", name="ps")
                pt = ps.bitcast(bf16)
                nc.tensor.transpose(
                    pt[0:D, 0:P], qn4[:, st, 2 * hp, 0:D], ident)
                nc.tensor.transpose(
                    pt[D:P, 0:P], qn4[:, st, 2 * hp + 1, 0:D], ident)
                nc.vector.tensor_copy(
                    out=qT[:, hp, st * P:(st + 1) * P], in_=pt[:, 0:P])

        def x_group(cid):
            # exchange chunk cid of attnT with the pair peer via AllGather;
            # both contributions land in attnF in fixed head order 0..15, so
            # the program stays uniform. Post-CC DMAs ride the gpsimd queue.
            j0, w = CHUNKS[cid]
            base, CHc = j0 * P, w * P
            # per-head-pair bounce writes: hp0-2's rows stream out while the
            # last head pair's normalization finishes
            bxi_r = bxi[cid].rearrange("(hp p) q -> p hp q", p=P)
            for hp in range(HP):
                nc.sync.dma_start(
                    out=bxi_r[:, hp, :],
                    in_=attnT[:, hp, base:base + CHc])
            nc.gpsimd.collective_compute(
                "AllGather", mybir.AluOpType.bypass,
                replica_groups=groups,
                ins=[bxi[cid].opt()], outs=[bxo[cid].opt()],
            )
            nc.gpsimd.dma_start(
                out=attnF[:, :, base:base + CHc],
                in_=bxo[cid].rearrange("(s p) q -> p s q", p=P))

        # NOTE: splitting the last chunk's exchange into two hp-half
        # AllGathers (3a mid-stream, 3b tail) was tried and REGRESSED
        # (+66us): each pair-AG carries a large fixed NRT cost and the
        # serial gpsimd queue compounds it.
        def o_group(cid, et):
            # output projection for chunk cid, e-tile et of my 512 columns,
            # over all 16 heads (attnF), bias added on the ACT-engine evict
            j0, w = CHUNKS[cid]
            base, CHc = j0 * P, w * P
            ps = pproj.tile([P, CH], f32, tag="ps", name="ps")
            for cp in range(2 * HP):
                nc.tensor.matmul(
                    ps[:, 0:CHc],
                    wo[:, cp, et * P:(et + 1) * P],
                    attnF[:, cp, base:base + CHc],
                    start=(cp == 0),
                    stop=(cp == 2 * HP - 1),
                )
            ot = ostp.tile([P, CH], bf16, tag="ot")
            nc.scalar.activation(out=ot[:, 0:CHc], in_=ps[:, 0:CHc],
                                 func=Ident, bias=bo_sb[:, et:et + 1])
            nc.sync.dma_start(
                out=out_d[et * P:(et + 1) * P, base:base + CHc],
                in_=ot[:, 0:CHc])

        # ---- prefix: projections needed by the qc0 attention units ----
        for st in range(4):
            qn_group(st)
        for hp in range(HP):
            k_group(hp, 0)
        for hp in range(HP):
            t_group(hp, 0)

        # ---- attention, qc-outer ----
        pending = []

        def flush_pending(use_pav=False):
            # NOTE: a [1,CH] reciprocal + partition_broadcast multiply was
            # tried to avoid the PE broadcast matmul, but DVE rejects
            # zero-step partition APs; the ones-outer-product stays.
            for (php, pcid, stgs) in pending:
                pj0, pw = CHUNKS[pcid]
                pbase, pCHc = pj0 * P, pw * P
                for half in range(2):
                    if use_pav:
                        rb = pav.tile([P, CH], f32,
                                      tag=("pvA" if half == 0 else "pvB"),
                                      name="rb")
                    else:
                        rb = pproj.tile([P, CH], f32, tag="ps", name="rb")
                    nc.tensor.matmul(rb[:, 0:pCHc], ones128[0:1, :],
                                     stgs[half][:, 0:pCHc],
                                     start=True, stop=True)
                    rcp = stgp.tile([P, CH], f32, tag="rbs", bufs=1)
                    nc.vector.reciprocal_approx_fast(out=rcp[:, 0:pCHc],
                                                     in_=rb[:, 0:pCHc])
                    dst = attnT[half * D:(half + 1) * D, php,
                                pbase:pbase + pCHc]
                    nc.vector.tensor_tensor(out=dst, in0=dst,
                                            in1=rcp[half * D:(half + 1) * D,
                                                    0:pCHc],
                                            op=mybir.AluOpType.mult)
            pending.clear()

        # chunk order 0,2,1,3,4: each chunk's AllGather exchange is emitted
        # one stream after its normalization and consumed (by the output
        # projection) later still, hiding the collective latency; only the
        # final 256-col chunk's exchange is tail-serial.
        qorder = [0, 2, 1, 3]
        NE = EH // P  # my output e-tiles (4)
        stream_tasks = {
            0: ([lambda st=st: qn_group(st) for st in range(4, 16)]
                + [lambda hp=hp, sc=sc: k_group(hp, sc)
                   for sc in (1, 2) for hp in range(HP)]
                + [lambda hp=hp: t_group(hp, 2) for hp in range(HP)]),
            2: ([lambda: x_group(0)]
                + [lambda hp=hp: t_group(hp, 1) for hp in range(HP)]),
            1: ([lambda: x_group(2)]
                + [lambda hp=hp: t_group(hp, 3) for hp in range(HP)]
                + [lambda hp=hp: k_group(hp, 3) for hp in range(HP)]
                + [lambda et=et: o_group(0, et) for et in range(NE)]),
            3: ([lambda: x_group(1)]
                + [lambda et=et: o_group(2, et) for et in range(NE)]),
        }

        for cid in qorder:
            j0, w = CHUNKS[cid]
            base, CHc = j0 * P, w * P
            tasks = stream_tasks[cid]
            ti = 0
            t_max = j0 + w - 1
            # stream 0 carries 24 projection tasks over only 16 short units:
            # dole several per unit (instead of one, which bunched 20 of
            # them into a solid post-stream block with poorer pacing)
            tstart = 2 if cid == 0 else 3
            slots = HP * max(1, t_max + 1 - tstart)
            tspace = max(1, slots // max(1, len(tasks)))
            tper = -(-len(tasks) // slots)  # ceil: tasks per eligible slot
            for hp in range(HP):
                hA, hB = 2 * hp, 2 * hp + 1
                pvA = pav.tile([P, CH], f32, tag="pvA")
                pvB = pav.tile([P, CH], f32, tag="pvB")
                # software pipeline: attnV for unit t is emitted during unit
                # t+1, after the next scores+exp have been issued, so the PE
                # never sits in the scores->exp->attnV dependency chain.
                pipe = None

                def attn_v(ex, t, qoff):
                    nc.tensor.matmul(
                        pvA[0:D + 1, qoff:CHc],
                        qn[:, t, hA * (D + 1):(hA + 1) * (D + 1)],
                        ex[:, 0, qoff:CHc],
                        start=(t == 0),
                        stop=(t == t_max),
                    )
                    nc.tensor.matmul(
                        pvB[0:D + 1, qoff:CHc],
                        qn[:, t, hB * (D + 1):(hB + 1) * (D + 1)],
                        ex[:, 1, qoff:CHc],
                        start=(t == 0),
                        stop=(t == t_max),
                    )

                for t in range(t_max + 1):
                    if t == 2:
                        flush_pending()
                    jloc = max(0, t - j0)
                    qoff = jloc * P
                    sc_t = psc.tile([P, 2, CH], f32, tag="sc_t")
                    nc.tensor.matmul(
                        sc_t[:, 0, qoff:CHc],
                        kT[0:D, hp, t * P:(t + 1) * P],
                        qT[0:D, hp, base + qoff:base + CHc],
                        start=True, stop=True,
                    )
                    nc.tensor.matmul(
                        sc_t[:, 1, qoff:CHc],
                        kT[D:P, hp, t * P:(t + 1) * P],
                        qT[D:P, hp, base + qoff:base + CHc],
                        start=True, stop=True,
                    )
                    ex = expp.tile([P, 2, CH], bf16)
                    nc.scalar.activation(
                        out=ex[:, :, qoff:CHc],
                        in_=sc_t[:, :, qoff:CHc],
                        func=Exp,
                        scale=scale,
                    )
                    if t >= j0:
                        # frontier 128-block: triu multiply (vector, NOT
                        # gpsimd: the collectives block the gpsimd queue)
                        for h2 in range(2):
                            nc.vector.tensor_mul(
                                out=ex[:, h2, qoff:qoff + P],
                                in0=ex[:, h2, qoff:qoff + P],
                                in1=tri,
                            )
                    if pipe is not None:
                        attn_v(*pipe)
                    pipe = (ex, t, qoff)
                    if (t >= tstart and (t - tstart) % tspace == 0
                            and ti < len(tasks)):
                        for _ in range(tper):
                            if ti < len(tasks):
                                tasks[ti]()
                                ti += 1

                attn_v(*pipe)
                # evict unnormalized attn + rowsum row; queue normalization
                stgs = []
                for pv, half in ((pvA, 0), (pvB, 1)):
                    stg = stgp.tile([1, CH], bf16, tag="stg", bufs=2)
                    nc.vector.tensor_copy(out=stg[:, 0:CHc],
                                          in_=pv[D:D + 1, 0:CHc])
                    nc.vector.tensor_copy(
                        out=attnT[half * D:(half + 1) * D, hp,
                                  base:base + CHc],
                        in_=pv[0:D, 0:CHc],
                    )
                    stgs.append(stg)
                pending.append((hp, cid, stgs))
            while ti < len(tasks):
                tasks[ti]()
                ti += 1

        # flush the last pending normalization into spare pav banks, then
        # tail: final 256-col exchange; chunk 3's output projection and warm
        # matmuls fill the PE while the last AllGather is in flight
        wdst2 = psc.tile([P, 2, CH], f32, tag="sc_t")
        for _ in range(4):
            nc.tensor.matmul(wdst2[:, 0, 0:256], wsrc[:, 0:P], wsrc,
                             start=True, stop=True)
        flush_pending(use_pav=True)
        x_group(3)
        for et in range(NE):
            o_group(1, et)
        # 60 warm matmuls: enough to hold the clock at 8/8 through most of
        # the tail AllGather; more (120 tried) overshoots the collective's
        # completion and delays o_group(3) behind the in-order PE queue.
        wdst3 = psc.tile([P, 2, CH], f32, tag="sc_t", name="wdst3")
        for _ in range(60):
            nc.tensor.matmul(wdst3[:, 0, 0:256], wsrc[:, 0:P], wsrc,
                             start=True, stop=True)
        for et in range(NE):
            o_group(3, et)

    nc.finalize()
    return nc


def _prep_inputs(x, Wk, Wq, Wo, bo, n_cores=NCORES):
    """Per-core input maps: batch = c//2, head half = c%2 (all bf16).

    wq/wk columns select the core's 8 heads; wo columns select the core's
    512 OUTPUT dims (e-split outproj over all 16 heads via the exchange).
    """
    b, s, e = x.shape
    P = 128
    EH = e // 2
    wqT = np.ascontiguousarray(Wq.T).astype(BF16)
    wkT = np.ascontiguousarray(Wk.T).astype(BF16)
    woT = np.ascontiguousarray(Wo.T).astype(BF16)
    consts = np.concatenate(
        [np.eye(P, dtype=np.float32),
         np.triu(np.ones((P, P), dtype=np.float32))], axis=1).astype(BF16)
    in_maps = []
    for c in range(n_cores):
        bi, hh = c // 2, c % 2
        xT = np.ascontiguousarray(x[bi].T).astype(BF16)
        bo_col = np.ascontiguousarray(
            bo[hh * EH:(hh + 1) * EH].reshape(EH // P, P).T
        ).astype(np.float32)
        in_maps.append({
            "xT": xT,
            "wqT": np.ascontiguousarray(wqT[:, hh * EH:(hh + 1) * EH]),
            "wkT": np.ascontiguousarray(wkT[:, hh * EH:(hh + 1) * EH]),
            "woT": np.ascontiguousarray(woT[:, hh * EH:(hh + 1) * EH]),
            "bo": bo_col,
            "consts": consts,
        })
    return in_maps


def kernel(x, Wk, Wq, Wv, Wo, bo):
    from concourse import bass_utils

    x = np.asarray(x, dtype=np.float32)
    Wk = np.asarray(Wk, dtype=np.float32)
    Wq = np.asarray(Wq, dtype=np.float32)
    Wo = np.asarray(Wo, dtype=np.float32)
    bo = np.asarray(bo, dtype=np.float32)
    b, s, e = x.shape
    key = (s, e, H)
    if key not in _CACHE:
        _CACHE[key] = _build_program(s, e, H)
    nc = _CACHE[key]
    in_maps = _prep_inputs(x, Wk, Wq, Wo, bo)
    res = bass_utils.run_bass_kernel_spmd(nc, in_maps, list(range(NCORES)))
    out = np.empty((b, s, e), dtype=np.float32)
    EH = e // 2
    for c in range(NCORES):
        bi, hh = c // 2, c % 2
        oc = np.asarray(res.results[c]["out"], dtype=np.float32)  # [EH, S]
        out[bi, :, hh * EH:(hh + 1) * EH] = oc.T
    return out


if __name__ == "__main__":
    nc = _build_program(S, E, H)
    print("built ok")


# revision 58
# speedup vs baseline: 1.2174x; 1.0124x over previous
# Multi-head masked attention (V = Q source quirk; Wv unused) on 8 TRN2 NeuronCores.
#
# Sharding: 8 cores = 4 batches x 2 head-halves (tensor parallel). Core c
# handles batch b = c//2 and heads hh*8..hh*8+7 (hh = c%2), for ALL queries.
# Each core projects K^T and Q-natural (= V) for its own 8 heads only (no
# duplicated projection work across the pair, unlike a query split), derives
# Q^T from Q-natural via PE transposes (4x cheaper than re-projecting), and
# runs causal attention for its heads over all 2048 queries. The pair then
# exchanges normalized attention outputs per query-chunk with pairwise
# AllGather collectives (chosen over ReduceScatter: ~16us vs ~45us per op on
# this NRT), after which each core runs the output projection over all 16
# heads for ITS 512 output columns (Wo column-half + bias live in the
# per-core input data) and writes out^T[my_e, all_q] directly. The program
# is fully SPMD-uniform: head/e-column assignment is carried by input data,
# and both pair contributions come back from the AllGather in fixed device
# order, so no rank-dependent addressing exists anywhere.
#
# Layouts (per core, bf16 matmul operands, fp32 PSUM accumulation):
#   kT  [128=d-in-pair, HP=4, S]   scores lhsT  (head even: partitions 0-63)
#   qT  [128=d-in-pair, HP, S]     scores rhs (from PE transposes of qn)
#   qn  [128=k-in-tile, S/128, 8*(D+1)]  attnV lhsT; col D of each head slot
#                                  is a ones column -> PSUM row 64 accumulates
#                                  the softmax denominator for free.
#   scores computed transposed (scoresT[k, q] = K @ Q^T); causal masking via
#   column-trimmed ranges + one triu multiply on the frontier 128-block.
#
# Schedule: chunk-outer / head-pair-inner attention in chunk order 0,2,1,3
# so each chunk's AllGather is emitted one stream after its normalization
# and consumed one stream later still. attnV for unit t is emitted during
# unit t+1 (software pipeline) so the PE never waits in the
# scores->exp->attnV chain; projection/outproj/exchange groups are doled out
# between attention units to keep the PE busy while the ACT engine (exp)
# catches up. Only the last chunk's exchange is tail-serial; chunk 1's
# output projection and warm matmuls (which also hold the HAM clock at 8/8)
# fill that window.

import sys

for _p in ("/opt/trn_rl_repo",):
    if _p not in sys.path:
        sys.path.append(_p)

import numpy as np
import ml_dtypes

BF16 = ml_dtypes.bfloat16

B, S, E, H = 4, 2048, 1024, 16
D = E // H
NCORES = 8
NH = H // 2          # local heads per core
HP = NH // 2         # local head pairs

_CACHE = {}


def _build_program(S, E, H, n_cores=NCORES):
    import concourse.bass as bass
    import concourse.mybir as mybir
    import concourse.tile as tile
    from concourse import bacc
    from contextlib import ExitStack

    P = 128
    D = E // H
    NH = H // 2
    HP = NH // 2
    assert D == 64 and S % 512 == 0 and E % P == 0
    S_t = S // P          # seq tiles (16)
    E_t = E // P          # embed tiles (8)
    EH = NH * D           # own hidden dims (512)
    CH = 512              # q chunk
    spc = CH // P         # subtiles per chunk (4)
    n_ch = S // CH        # chunks (4)
    Lq = S // 2           # output rows per core
    f32 = mybir.dt.float32
    bf16 = mybir.dt.bfloat16
    Exp = mybir.ActivationFunctionType.Exp
    Ident = mybir.ActivationFunctionType.Identity
    scale = 1.0 / float(np.sqrt(E))
    groups = [[2 * i, 2 * i + 1] for i in range(n_cores // 2)]

    nc = bacc.Bacc(
        "TRN2", target_bir_lowering=False, debug=False, num_devices=n_cores
    )

    xT_d = nc.dram_tensor("xT", [E, S], bf16, kind="ExternalInput").ap()
    wqT_d = nc.dram_tensor("wqT", [E, EH], bf16, kind="ExternalInput").ap()
    wkT_d = nc.dram_tensor("wkT", [E, EH], bf16, kind="ExternalInput").ap()
    # full hidden rows x my 512 output columns (e-split output projection)
    woT_d = nc.dram_tensor("woT", [E, EH], bf16, kind="ExternalInput").ap()
    bo_d = nc.dram_tensor("bo", [P, EH // P], f32, kind="ExternalInput").ap()
    consts_d = nc.dram_tensor("consts", [P, 2 * P], bf16,
                              kind="ExternalInput").ap()
    # transposed output: my 512 e-columns for ALL queries
    out_d = nc.dram_tensor("out", [EH, S], bf16, kind="ExternalOutput").ap()

    with tile.TileContext(nc) as tc, ExitStack() as ctx:
        main = ctx.enter_context(tc.tile_pool(name="main", bufs=1))
        expp = ctx.enter_context(tc.tile_pool(name="expp", bufs=3))
        stgp = ctx.enter_context(tc.tile_pool(name="stgp", bufs=2))
        ostp = ctx.enter_context(tc.tile_pool(name="ostp", bufs=3))
        dram = ctx.enter_context(tc.tile_pool(name="dram", bufs=1,
                                              space="DRAM"))

        xTa = main.tile([P, E_t, S], bf16, tag="xTa", name="xTa")
        wqf = main.tile([P, E_t, EH], bf16)
        wkf = main.tile([P, E_t, EH], bf16)
        wo = main.tile([P, E_t, EH], bf16)
        qn = main.tile([P, S_t, NH * (D + 1)], bf16)
        kT = main.tile([P, HP, S], bf16)
        qT = main.tile([P, HP, S], bf16)
        attnT = main.tile([P, HP, S], bf16)
        # all 16 heads' attn (own + peer, head order 0..15), via AllGather
        attnF = main.tile([P, 2 * HP, S], bf16)
        consts = main.tile([P, 2 * P], bf16)
        bo_sb = main.tile([P, EH // P], f32)
        ones128 = main.tile([1, P], bf16)
        wsrc = main.tile([P, 256], bf16)

        # attention chunks as (first q-tile, width in tiles). NOTE: splitting
        # the last chunk into two 256-col chunks was tried and REGRESSED
        # (+53us): the extra per-unit overhead and three back-to-back
        # AllGathers on the serial gpsimd queue cost more than the smaller
        # tail exchange saved.
        CHUNKS = [(0, 4), (4, 4), (8, 4), (12, 4)]
        bxi = [dram.tile([HP * P, w * P], bf16, tag=f"bxi{c}", name=f"bxi{c}")
               for c, (j0, w) in enumerate(CHUNKS)]
        bxo = [dram.tile([2 * HP * P, w * P], bf16, tag=f"bxo{c}",
                         name=f"bxo{c}") for c, (j0, w) in enumerate(CHUNKS)]


        nc.vector.memset(ones128, 1.0)
        nc.vector.memset(wsrc, 0.0)

        ident = consts[:, 0:P]
        tri = consts[:, P:2 * P]
        qn4 = qn.rearrange("p t (h c) -> p t h c", c=D + 1)
        xT_r = xT_d.rearrange("(t p) s -> p t s", p=P)
        wq_r = wqT_d.rearrange("(t p) d -> p t d", p=P)
        wk_r = wkT_d.rearrange("(t p) d -> p t d", p=P)
        wo_r = woT_d.rearrange("(t p) e -> p t e", p=P)

        # ---- DMA issue order: first-needed first. Per-e-tile configs: one
        # big multi-tile config per tensor was tried and REGRESSED (+58us) —
        # small configs spread across more parallel DMA queues.
        for e in range(E_t):
            nc.sync.dma_start(out=wqf[:, e, :], in_=wq_r[:, e, :])
        for e in range(E_t):
            nc.sync.dma_start(out=xTa[:, e, 0:CH], in_=xT_r[:, e, 0:CH])
        for e in range(E_t):
            nc.sync.dma_start(out=wkf[:, e, :], in_=wk_r[:, e, :])
        nc.sync.dma_start(out=consts, in_=consts_d)
        for sc in range(1, n_ch):
            for e in range(E_t):
                nc.sync.dma_start(
                    out=xTa[:, e, sc * CH:(sc + 1) * CH],
                    in_=xT_r[:, e, sc * CH:(sc + 1) * CH],
                )
        nc.sync.dma_start(out=bo_sb, in_=bo_d)
        for e in range(E_t):
            nc.sync.dma_start(out=wo[:, e, :], in_=wo_r[:, e, :])

        # ---- PE warmup: keep HAM at 8/8 while the initial DMAs stream ----
        with tc.tile_pool(name="wps", bufs=1, space="PSUM") as wpsp:
            wdst = wpsp.tile([P, 256], f32)
            for _ in range(68):
                nc.tensor.matmul(wdst, wsrc[:, 0:P], wsrc, start=True,
                                 stop=True)

        pproj = ctx.enter_context(
            tc.tile_pool(name="pproj", bufs=2, space="PSUM"))
        psc = ctx.enter_context(
            tc.tile_pool(name="psc", bufs=2, space="PSUM"))
        pav = ctx.enter_context(
            tc.tile_pool(name="pav", bufs=1, space="PSUM"))

        # ---- task groups ----
        def qn_group(st):
            ps = pproj.tile([P, CH], f32, tag="ps", name="ps")
            for e in range(E_t):
                nc.tensor.matmul(
                    ps,
                    xTa[:, e, st * P:(st + 1) * P],
                    wqf[:, e, :],
                    start=(e == 0),
                    stop=(e == E_t - 1),
                )
            nc.vector.tensor_copy(
                out=qn4[:, st, :, 0:D],
                in_=ps.rearrange("p (h c) -> p h c", c=D),
            )
            nc.gpsimd.memset(qn4[:, st, :, D:D + 1], 1.0)

        def k_group(hp, sc):
            ps = pproj.tile([P, CH], f32, tag="ps", name="ps")
            for e in range(E_t):
                nc.tensor.matmul(
                    ps,
                    wkf[:, e, hp * P:(hp + 1) * P],
                    xTa[:, e, sc * CH:(sc + 1) * CH],
                    start=(e == 0),
                    stop=(e == E_t - 1),
                )
            nc.vector.tensor_copy(out=kT[:, hp, sc * CH:(sc + 1) * CH],
                                  in_=ps)

        def t_group(hp, qc):
            # transpose qn d-blocks of 4 seq tiles into qT for one head pair.
            # (A single strided-lhsT transpose covering both heads was tried;
            # walrus rejects the strided transpose weight AP.)
            for st in range(4 * qc, 4 * qc + 4):
                ps = pproj.tile([P, CH], f32, tag="ps", name="ps")
                pt = ps.bitcast(bf16)
                nc.tensor.transpose(
                    pt[0:D, 0:P], qn4[:, st, 2 * hp, 0:D], ident)
                nc.tensor.transpose(
                    pt[D:P, 0:P], qn4[:, st, 2 * hp + 1, 0:D], ident)
                nc.vector.tensor_copy(
                    out=qT[:, hp, st * P:(st + 1) * P], in_=pt[:, 0:P])

        def x_group(cid):
            # exchange chunk cid of attnT with the pair peer via AllGather;
            # both contributions land in attnF in fixed head order 0..15, so
            # the program stays uniform. Post-CC DMAs ride the gpsimd queue.
            j0, w = CHUNKS[cid]
            base, CHc = j0 * P, w * P
            # per-head-pair bounce writes: hp0-2's rows stream out while the
            # last head pair's normalization finishes
            bxi_r = bxi[cid].rearrange("(hp p) q -> p hp q", p=P)
            for hp in range(HP):
                nc.sync.dma_start(
                    out=bxi_r[:, hp, :],
                    in_=attnT[:, hp, base:base + CHc])
            nc.gpsimd.collective_compute(
                "AllGather", mybir.AluOpType.bypass,
                replica_groups=groups,
                ins=[bxi[cid].opt()], outs=[bxo[cid].opt()],
            )
            nc.gpsimd.dma_start(
                out=attnF[:, :, base:base + CHc],
                in_=bxo[cid].rearrange("(s p) q -> p s q", p=P))

        # NOTE: splitting the last chunk's exchange into two hp-half
        # AllGathers (3a mid-stream, 3b tail) was tried and REGRESSED
        # (+66us): each pair-AG carries a large fixed NRT cost and the
        # serial gpsimd queue compounds it.
        def o_group(cid, et):
            # output projection for chunk cid, e-tile et of my 512 columns,
            # over all 16 heads (attnF), bias added on the ACT-engine evict
            j0, w = CHUNKS[cid]
            base, CHc = j0 * P, w * P
            ps = pproj.tile([P, CH], f32, tag="ps", name="ps")
            for cp in range(2 * HP):
                nc.tensor.matmul(
                    ps[:, 0:CHc],
                    wo[:, cp, et * P:(et + 1) * P],
                    attnF[:, cp, base:base + CHc],
                    start=(cp == 0),
                    stop=(cp == 2 * HP - 1),
                )
            ot = ostp.tile([P, CH], bf16, tag="ot")
            nc.scalar.activation(out=ot[:, 0:CHc], in_=ps[:, 0:CHc],
                                 func=Ident, bias=bo_sb[:, et:et + 1])
            nc.sync.dma_start(
                out=out_d[et * P:(et + 1) * P, base:base + CHc],
                in_=ot[:, 0:CHc])

        # ---- prefix: projections needed by the qc0 attention units ----
        for st in range(4):
            qn_group(st)
        for hp in range(HP):
            k_group(hp, 0)
        for hp in range(HP):
            t_group(hp, 0)

        # ---- attention, qc-outer ----
        pending = []

        def flush_pending(use_pav=False):
            # NOTE: a [1,CH] reciprocal + partition_broadcast multiply was
            # tried to avoid the PE broadcast matmul, but DVE rejects
            # zero-step partition APs; the ones-outer-product stays.
            for (php, pcid, stgs) in pending:
                pj0, pw = CHUNKS[pcid]
                pbase, pCHc = pj0 * P, pw * P
                for half in range(2):
                    if use_pav:
                        rb = pav.tile([P, CH], f32,
                                      tag=("pvA" if half == 0 else "pvB"),
                                      name="rb")
                    else:
                        rb = pproj.tile([P, CH], f32, tag="ps", name="rb")
                    nc.tensor.matmul(rb[:, 0:pCHc], ones128[0:1, :],
                                     stgs[half][:, 0:pCHc],
                                     start=True, stop=True)
                    rcp = stgp.tile([P, CH], f32, tag="rbs", bufs=1)
                    nc.vector.reciprocal_approx_fast(out=rcp[:, 0:pCHc],
                                                     in_=rb[:, 0:pCHc])
                    dst = attnT[half * D:(half + 1) * D, php,
                                pbase:pbase + pCHc]
                    nc.vector.tensor_tensor(out=dst, in0=dst,
                                            in1=rcp[half * D:(half + 1) * D,
                                                    0:pCHc],
                                            op=mybir.AluOpType.mult)
            pending.clear()

        # chunk order 0,2,1,3,4: each chunk's AllGather exchange is emitted
        # one stream after its normalization and consumed (by the output
        # projection) later still, hiding the collective latency; only the
        # final 256-col chunk's exchange is tail-serial.
        qorder = [0, 2, 1, 3]
        NE = EH // P  # my output e-tiles (4)
        stream_tasks = {
            0: ([lambda st=st: qn_group(st) for st in range(4, 16)]
                + [lambda hp=hp, sc=sc: k_group(hp, sc)
                   for sc in (1, 2) for hp in range(HP)]
                + [lambda hp=hp: t_group(hp, 2) for hp in range(HP)]),
            2: ([lambda hp=hp: t_group(hp, 1) for hp in range(HP)]
                + [lambda: x_group(0)]),
            1: ([lambda hp=hp: t_group(hp, 3) for hp in range(HP)]
                + [lambda hp=hp: k_group(hp, 3) for hp in range(HP)]
                + [lambda: x_group(2)]
                + [lambda et=et: o_group(0, et) for et in range(NE)]),
            3: ([lambda: x_group(1)]
                + [lambda et=et: o_group(2, et) for et in range(NE)]),
        }

        for cid in qorder:
            j0, w = CHUNKS[cid]
            base, CHc = j0 * P, w * P
            tasks = stream_tasks[cid]
            ti = 0
            t_max = j0 + w - 1
            # dole fill tasks from t=0 so the first units of each head pair
            # (exp-pipeline refill bubbles) stay covered. Safe for streams
            # 0/1/2 because their attnT-dependent tasks (x_group/o_group)
            # sit late enough in the list to land after the t==2 norm flush;
            # stream 3's first task is x_group(1), so it keeps tstart=3.
            tstart = 3 if cid == 3 else 0
            slots = HP * max(1, t_max + 1 - tstart)
            tspace = max(1, slots // max(1, len(tasks)))
            tper = -(-len(tasks) // slots)  # ceil: tasks per eligible slot
            for hp in range(HP):
                hA, hB = 2 * hp, 2 * hp + 1
                pvA = pav.tile([P, CH], f32, tag="pvA")
                pvB = pav.tile([P, CH], f32, tag="pvB")
                # software pipeline: attnV for unit t is emitted during unit
                # t+1, after the next scores+exp have been issued, so the PE
                # never sits in the scores->exp->attnV dependency chain.
                pipe = None

                def attn_v(ex, t, qoff):
                    nc.tensor.matmul(
                        pvA[0:D + 1, qoff:CHc],
                        qn[:, t, hA * (D + 1):(hA + 1) * (D + 1)],
                        ex[:, 0, qoff:CHc],
                        start=(t == 0),
                        stop=(t == t_max),
                    )
                    nc.tensor.matmul(
                        pvB[0:D + 1, qoff:CHc],
                        qn[:, t, hB * (D + 1):(hB + 1) * (D + 1)],
                        ex[:, 1, qoff:CHc],
                        start=(t == 0),
                        stop=(t == t_max),
                    )

                for t in range(t_max + 1):
                    if t == 2:
                        flush_pending()
                    jloc = max(0, t - j0)
                    qoff = jloc * P
                    sc_t = psc.tile([P, 2, CH], f32, tag="sc_t")
                    nc.tensor.matmul(
                        sc_t[:, 0, qoff:CHc],
                        kT[0:D, hp, t * P:(t + 1) * P],
                        qT[0:D, hp, base + qoff:base + CHc],
                        start=True, stop=True,
                    )
                    nc.tensor.matmul(
                        sc_t[:, 1, qoff:CHc],
                        kT[D:P, hp, t * P:(t + 1) * P],
                        qT[D:P, hp, base + qoff:base + CHc],
                        start=True, stop=True,
                    )
                    ex = expp.tile([P, 2, CH], bf16)
                    nc.scalar.activation(
                        out=ex[:, :, qoff:CHc],
                        in_=sc_t[:, :, qoff:CHc],
                        func=Exp,
                        scale=scale,
                    )
                    if t >= j0:
                        # frontier 128-block: triu multiply (vector, NOT
                        # gpsimd: the collectives block the gpsimd queue)
                        for h2 in range(2):
                            nc.vector.tensor_mul(
                                out=ex[:, h2, qoff:qoff + P],
                                in0=ex[:, h2, qoff:qoff + P],
                                in1=tri,
                            )
                    if pipe is not None:
                        attn_v(*pipe)
                    pipe = (ex, t, qoff)
                    if (t >= tstart and (t - tstart) % tspace == 0
                            and ti < len(tasks)):
                        for _ in range(tper):
                            if ti < len(tasks):
                                tasks[ti]()
                                ti += 1

                attn_v(*pipe)
                # evict unnormalized attn + rowsum row; queue normalization
                stgs = []
                for pv, half in ((pvA, 0), (pvB, 1)):
                    stg = stgp.tile([1, CH], bf16, tag="stg", bufs=2)
                    nc.vector.tensor_copy(out=stg[:, 0:CHc],
                                          in_=pv[D:D + 1, 0:CHc])
                    nc.vector.tensor_copy(
                        out=attnT[half * D:(half + 1) * D, hp,
                                  base:base + CHc],
                        in_=pv[0:D, 0:CHc],
                    )
                    stgs.append(stg)
                pending.append((hp, cid, stgs))
            while ti < len(tasks):
                tasks[ti]()
                ti += 1

        # flush the last pending normalization into spare pav banks, then
        # tail: final 256-col exchange; chunk 3's output projection and warm
        # matmuls fill the PE while the last AllGather is in flight
        wdst2 = psc.tile([P, 2, CH], f32, tag="sc_t")
        for _ in range(4):
            nc.tensor.matmul(wdst2[:, 0, 0:256], wsrc[:, 0:P], wsrc,
                             start=True, stop=True)
        flush_pending(use_pav=True)
        x_group(3)
        for et in range(NE):
            o_group(1, et)
        # 60 warm matmuls: enough to hold the clock at 8/8 through most of
        # the tail AllGather; more (120 tried) overshoots the collective's
        # completion and delays o_group(3) behind the in-order PE queue.
        wdst3 = psc.tile([P, 2, CH], f32, tag="sc_t", name="wdst3")
        for _ in range(60):
            nc.tensor.matmul(wdst3[:, 0, 0:256], wsrc[:, 0:P], wsrc,
                             start=True, stop=True)
        for et in range(NE):
            o_group(3, et)

    nc.finalize()
    return nc


def _prep_inputs(x, Wk, Wq, Wo, bo, n_cores=NCORES):
    """Per-core input maps: batch = c//2, head half = c%2 (all bf16).

    wq/wk columns select the core's 8 heads; wo columns select the core's
    512 OUTPUT dims (e-split outproj over all 16 heads via the exchange).
    """
    b, s, e = x.shape
    P = 128
    EH = e // 2
    wqT = np.ascontiguousarray(Wq.T).astype(BF16)
    wkT = np.ascontiguousarray(Wk.T).astype(BF16)
    woT = np.ascontiguousarray(Wo.T).astype(BF16)
    consts = np.concatenate(
        [np.eye(P, dtype=np.float32),
         np.triu(np.ones((P, P), dtype=np.float32))], axis=1).astype(BF16)
    in_maps = []
    for c in range(n_cores):
        bi, hh = c // 2, c % 2
        xT = np.ascontiguousarray(x[bi].T).astype(BF16)
        bo_col = np.ascontiguousarray(
            bo[hh * EH:(hh + 1) * EH].reshape(EH // P, P).T
        ).astype(np.float32)
        in_maps.append({
            "xT": xT,
            "wqT": np.ascontiguousarray(wqT[:, hh * EH:(hh + 1) * EH]),
            "wkT": np.ascontiguousarray(wkT[:, hh * EH:(hh + 1) * EH]),
            "woT": np.ascontiguousarray(woT[:, hh * EH:(hh + 1) * EH]),
            "bo": bo_col,
            "consts": consts,
        })
    return in_maps


def kernel(x, Wk, Wq, Wv, Wo, bo):
    from concourse import bass_utils

    x = np.asarray(x, dtype=np.float32)
    Wk = np.asarray(Wk, dtype=np.float32)
    Wq = np.asarray(Wq, dtype=np.float32)
    Wo = np.asarray(Wo, dtype=np.float32)
    bo = np.asarray(bo, dtype=np.float32)
    b, s, e = x.shape
    key = (s, e, H)
    if key not in _CACHE:
        _CACHE[key] = _build_program(s, e, H)
    nc = _CACHE[key]
    in_maps = _prep_inputs(x, Wk, Wq, Wo, bo)
    res = bass_utils.run_bass_kernel_spmd(nc, in_maps, list(range(NCORES)))
    out = np.empty((b, s, e), dtype=np.float32)
    EH = e // 2
    for c in range(NCORES):
        bi, hh = c // 2, c % 2
        oc = np.asarray(res.results[c]["out"], dtype=np.float32)  # [EH, S]
        out[bi, :, hh * EH:(hh + 1) * EH] = oc.T
    return out


if __name__ == "__main__":
    nc = _build_program(S, E, H)
    print("built ok")


# revision 61
# speedup vs baseline: 1.2661x; 1.0400x over previous
# Multi-head masked attention (V = Q source quirk; Wv unused) on 8 TRN2 NeuronCores.
#
# Sharding: 8 cores = 4 batches x 2 head-halves (tensor parallel). Core c
# handles batch b = c//2 and heads hh*8..hh*8+7 (hh = c%2), for ALL queries.
# Each core projects K^T and Q-natural (= V) for its own 8 heads only (no
# duplicated projection work across the pair, unlike a query split), derives
# Q^T from Q-natural via PE transposes (4x cheaper than re-projecting), and
# runs causal attention for its heads over all 2048 queries. The pair then
# exchanges normalized attention outputs per query-chunk with pairwise
# AllGather collectives (chosen over ReduceScatter: ~16us vs ~45us per op on
# this NRT), after which each core runs the output projection over all 16
# heads for ITS 512 output columns (Wo column-half + bias live in the
# per-core input data) and writes out^T[my_e, all_q] directly. The program
# is fully SPMD-uniform: head/e-column assignment is carried by input data,
# and both pair contributions come back from the AllGather in fixed device
# order, so no rank-dependent addressing exists anywhere.
#
# Layouts (per core, bf16 matmul operands, fp32 PSUM accumulation):
#   kT  [128=d-in-pair, HP=4, S]   scores lhsT  (head even: partitions 0-63)
#   qT  [128=d-in-pair, HP, S]     scores rhs (from PE transposes of qn)
#   qn  [128=k-in-tile, S/128, 8*(D+1)]  attnV lhsT; col D of each head slot
#                                  is a ones column -> PSUM row 64 accumulates
#                                  the softmax denominator for free.
#   scores computed transposed (scoresT[k, q] = K @ Q^T); causal masking via
#   column-trimmed ranges + one triu multiply on the frontier 128-block.
#
# Schedule: chunk-outer / head-pair-inner attention in chunk order 0,2,1,3
# so each chunk's AllGather is emitted one stream after its normalization
# and consumed one stream later still. attnV for unit t is emitted during
# unit t+1 (software pipeline) so the PE never waits in the
# scores->exp->attnV chain; projection/outproj/exchange groups are doled out
# between attention units to keep the PE busy while the ACT engine (exp)
# catches up. Only the last chunk's exchange is tail-serial; chunk 1's
# output projection and warm matmuls (which also hold the HAM clock at 8/8)
# fill that window.

import sys

for _p in ("/opt/trn_rl_repo",):
    if _p not in sys.path:
        sys.path.append(_p)

import numpy as np
import ml_dtypes

BF16 = ml_dtypes.bfloat16

B, S, E, H = 4, 2048, 1024, 16
D = E // H
NCORES = 8
NH = H // 2          # local heads per core
HP = NH // 2         # local head pairs

_CACHE = {}


def _build_program(S, E, H, n_cores=NCORES):
    import concourse.bass as bass
    import concourse.mybir as mybir
    import concourse.tile as tile
    from concourse import bacc
    from contextlib import ExitStack

    P = 128
    D = E // H
    NH = H // 2
    HP = NH // 2
    assert D == 64 and S % 512 == 0 and E % P == 0
    S_t = S // P          # seq tiles (16)
    E_t = E // P          # embed tiles (8)
    EH = NH * D           # own hidden dims (512)
    CH = 512              # q chunk
    spc = CH // P         # subtiles per chunk (4)
    n_ch = S // CH        # chunks (4)
    Lq = S // 2           # output rows per core
    f32 = mybir.dt.float32
    bf16 = mybir.dt.bfloat16
    Exp = mybir.ActivationFunctionType.Exp
    Ident = mybir.ActivationFunctionType.Identity
    scale = 1.0 / float(np.sqrt(E))
    groups = [[2 * i, 2 * i + 1] for i in range(n_cores // 2)]

    nc = bacc.Bacc(
        "TRN2", target_bir_lowering=False, debug=False, num_devices=n_cores
    )

    xT_d = nc.dram_tensor("xT", [E, S], bf16, kind="ExternalInput").ap()
    wqT_d = nc.dram_tensor("wqT", [E, EH], bf16, kind="ExternalInput").ap()
    wkT_d = nc.dram_tensor("wkT", [E, EH], bf16, kind="ExternalInput").ap()
    # full hidden rows x my 512 output columns (e-split output projection)
    woT_d = nc.dram_tensor("woT", [E, EH], bf16, kind="ExternalInput").ap()
    bo_d = nc.dram_tensor("bo", [P, EH // P], f32, kind="ExternalInput").ap()
    consts_d = nc.dram_tensor("consts", [P, 2 * P], bf16,
                              kind="ExternalInput").ap()
    # transposed output: my 512 e-columns for ALL queries
    out_d = nc.dram_tensor("out", [EH, S], bf16, kind="ExternalOutput").ap()

    with tile.TileContext(nc) as tc, ExitStack() as ctx:
        main = ctx.enter_context(tc.tile_pool(name="main", bufs=1))
        expp = ctx.enter_context(tc.tile_pool(name="expp", bufs=3))
        stgp = ctx.enter_context(tc.tile_pool(name="stgp", bufs=2))
        ostp = ctx.enter_context(tc.tile_pool(name="ostp", bufs=3))
        dram = ctx.enter_context(tc.tile_pool(name="dram", bufs=1,
                                              space="DRAM"))

        xTa = main.tile([P, E_t, S], bf16, tag="xTa", name="xTa")
        wqf = main.tile([P, E_t, EH], bf16)
        wkf = main.tile([P, E_t, EH], bf16)
        wo = main.tile([P, E_t, EH], bf16)
        qn = main.tile([P, S_t, NH * (D + 1)], bf16)
        kT = main.tile([P, HP, S], bf16)
        qT = main.tile([P, HP, S], bf16)
        attnT = main.tile([P, HP, S], bf16)
        # all 16 heads' attn (own + peer, head order 0..15), via AllGather
        attnF = main.tile([P, 2 * HP, S], bf16)
        consts = main.tile([P, 2 * P], bf16)
        bo_sb = main.tile([P, EH // P], f32)
        ones128 = main.tile([1, P], bf16)
        wsrc = main.tile([P, 256], bf16)

        # attention chunks as (first q-tile, width in tiles). NOTE: splitting
        # the last chunk into two 256-col chunks was tried and REGRESSED
        # (+53us): the extra per-unit overhead and three back-to-back
        # AllGathers on the serial gpsimd queue cost more than the smaller
        # tail exchange saved.
        CHUNKS = [(0, 4), (4, 4), (8, 4), (12, 4)]
        bxi = [dram.tile([HP * P, w * P], bf16, tag=f"bxi{c}", name=f"bxi{c}")
               for c, (j0, w) in enumerate(CHUNKS)]
        bxo = [dram.tile([2 * HP * P, w * P], bf16, tag=f"bxo{c}",
                         name=f"bxo{c}") for c, (j0, w) in enumerate(CHUNKS)]


        nc.vector.memset(ones128, 1.0)
        nc.vector.memset(wsrc, 0.0)

        ident = consts[:, 0:P]
        tri = consts[:, P:2 * P]
        qn4 = qn.rearrange("p t (h c) -> p t h c", c=D + 1)
        xT_r = xT_d.rearrange("(t p) s -> p t s", p=P)
        wq_r = wqT_d.rearrange("(t p) d -> p t d", p=P)
        wk_r = wkT_d.rearrange("(t p) d -> p t d", p=P)
        wo_r = woT_d.rearrange("(t p) e -> p t e", p=P)

        # ---- DMA issue order: first-needed first. Per-e-tile configs: one
        # big multi-tile config per tensor was tried and REGRESSED (+58us) —
        # small configs spread across more parallel DMA queues.
        for e in range(E_t):
            nc.sync.dma_start(out=wqf[:, e, :], in_=wq_r[:, e, :])
        for e in range(E_t):
            nc.sync.dma_start(out=xTa[:, e, 0:CH], in_=xT_r[:, e, 0:CH])
        for e in range(E_t):
            nc.sync.dma_start(out=wkf[:, e, :], in_=wk_r[:, e, :])
        nc.sync.dma_start(out=consts, in_=consts_d)
        for sc in range(1, n_ch):
            for e in range(E_t):
                nc.sync.dma_start(
                    out=xTa[:, e, sc * CH:(sc + 1) * CH],
                    in_=xT_r[:, e, sc * CH:(sc + 1) * CH],
                )
        nc.sync.dma_start(out=bo_sb, in_=bo_d)
        for e in range(E_t):
            nc.sync.dma_start(out=wo[:, e, :], in_=wo_r[:, e, :])

        # ---- PE warmup: keep HAM at 8/8 while the initial DMAs stream ----
        with tc.tile_pool(name="wps", bufs=1, space="PSUM") as wpsp:
            wdst = wpsp.tile([P, 256], f32)
            for _ in range(68):
                nc.tensor.matmul(wdst, wsrc[:, 0:P], wsrc, start=True,
                                 stop=True)

        pproj = ctx.enter_context(
            tc.tile_pool(name="pproj", bufs=2, space="PSUM"))
        psc = ctx.enter_context(
            tc.tile_pool(name="psc", bufs=2, space="PSUM"))
        pav = ctx.enter_context(
            tc.tile_pool(name="pav", bufs=1, space="PSUM"))

        # ---- task groups ----
        def qn_group(st):
            ps = pproj.tile([P, CH], f32, tag="ps", name="ps")
            for e in range(E_t):
                nc.tensor.matmul(
                    ps,
                    xTa[:, e, st * P:(st + 1) * P],
                    wqf[:, e, :],
                    start=(e == 0),
                    stop=(e == E_t - 1),
                )
            nc.vector.tensor_copy(
                out=qn4[:, st, :, 0:D],
                in_=ps.rearrange("p (h c) -> p h c", c=D),
            )
            nc.gpsimd.memset(qn4[:, st, :, D:D + 1], 1.0)

        def k_group(hp, sc):
            ps = pproj.tile([P, CH], f32, tag="ps", name="ps")
            for e in range(E_t):
                nc.tensor.matmul(
                    ps,
                    wkf[:, e, hp * P:(hp + 1) * P],
                    xTa[:, e, sc * CH:(sc + 1) * CH],
                    start=(e == 0),
                    stop=(e == E_t - 1),
                )
            nc.vector.tensor_copy(out=kT[:, hp, sc * CH:(sc + 1) * CH],
                                  in_=ps)

        def t_group(hp, qc):
            # transpose qn d-blocks of 4 seq tiles into qT for one head pair.
            # (A single strided-lhsT transpose covering both heads was tried;
            # walrus rejects the strided transpose weight AP.)
            for st in range(4 * qc, 4 * qc + 4):
                ps = pproj.tile([P, CH], f32, tag="ps", name="ps")
                pt = ps.bitcast(bf16)
                nc.tensor.transpose(
                    pt[0:D, 0:P], qn4[:, st, 2 * hp, 0:D], ident)
                nc.tensor.transpose(
                    pt[D:P, 0:P], qn4[:, st, 2 * hp + 1, 0:D], ident)
                nc.vector.tensor_copy(
                    out=qT[:, hp, st * P:(st + 1) * P], in_=pt[:, 0:P])

        def x_group(cid):
            # exchange chunk cid of attnT with the pair peer via AllGather;
            # both contributions land in attnF in fixed head order 0..15, so
            # the program stays uniform. Post-CC DMAs ride the gpsimd queue.
            j0, w = CHUNKS[cid]
            base, CHc = j0 * P, w * P
            # per-head-pair bounce writes: hp0-2's rows stream out while the
            # last head pair's normalization finishes
            bxi_r = bxi[cid].rearrange("(hp p) q -> p hp q", p=P)
            for hp in range(HP):
                nc.sync.dma_start(
                    out=bxi_r[:, hp, :],
                    in_=attnT[:, hp, base:base + CHc])
            nc.gpsimd.collective_compute(
                "AllGather", mybir.AluOpType.bypass,
                replica_groups=groups,
                ins=[bxi[cid].opt()], outs=[bxo[cid].opt()],
            )
            nc.gpsimd.dma_start(
                out=attnF[:, :, base:base + CHc],
                in_=bxo[cid].rearrange("(s p) q -> p s q", p=P))

        # NOTE: splitting the last chunk's exchange into two hp-half
        # AllGathers (3a mid-stream, 3b tail) was tried and REGRESSED
        # (+66us): each pair-AG carries a large fixed NRT cost and the
        # serial gpsimd queue compounds it.
        def o_group(cid, et):
            # output projection for chunk cid, e-tile et of my 512 columns,
            # over all 16 heads (attnF), bias added on the ACT-engine evict
            j0, w = CHUNKS[cid]
            base, CHc = j0 * P, w * P
            ps = pproj.tile([P, CH], f32, tag="ps", name="ps")
            for cp in range(2 * HP):
                nc.tensor.matmul(
                    ps[:, 0:CHc],
                    wo[:, cp, et * P:(et + 1) * P],
                    attnF[:, cp, base:base + CHc],
                    start=(cp == 0),
                    stop=(cp == 2 * HP - 1),
                )
            ot = ostp.tile([P, CH], bf16, tag="ot")
            # bias-add on the vector engine: the ACT engine is ~84% busy
            # with exp in the last attention stream
            nc.vector.tensor_scalar_add(out=ot[:, 0:CHc], in0=ps[:, 0:CHc],
                                        scalar1=bo_sb[:, et:et + 1])
            nc.sync.dma_start(
                out=out_d[et * P:(et + 1) * P, base:base + CHc],
                in_=ot[:, 0:CHc])

        # ---- prefix: projections needed by the qc0 attention units ----
        for st in range(4):
            qn_group(st)
        for hp in range(HP):
            k_group(hp, 0)
        for hp in range(HP):
            t_group(hp, 0)

        # ---- attention, qc-outer ----
        pending = []

        def flush_pending(use_pav=False):
            # NOTE: a [1,CH] reciprocal + partition_broadcast multiply was
            # tried to avoid the PE broadcast matmul, but DVE rejects
            # zero-step partition APs; the ones-outer-product stays.
            for (php, pcid, stgs) in pending:
                pj0, pw = CHUNKS[pcid]
                pbase, pCHc = pj0 * P, pw * P
                for half in range(2):
                    if use_pav:
                        rb = pav.tile([P, CH], f32,
                                      tag=("pvA" if half == 0 else "pvB"),
                                      name="rb")
                    else:
                        rb = pproj.tile([P, CH], f32, tag="ps", name="rb")
                    nc.tensor.matmul(rb[:, 0:pCHc], ones128[0:1, :],
                                     stgs[half][:, 0:pCHc],
                                     start=True, stop=True)
                    rcp = stgp.tile([P, CH], f32, tag="rbs", bufs=1)
                    nc.vector.reciprocal_approx_fast(out=rcp[:, 0:pCHc],
                                                     in_=rb[:, 0:pCHc])
                    dst = attnT[half * D:(half + 1) * D, php,
                                pbase:pbase + pCHc]
                    nc.vector.tensor_tensor(out=dst, in0=dst,
                                            in1=rcp[half * D:(half + 1) * D,
                                                    0:pCHc],
                                            op=mybir.AluOpType.mult)
            pending.clear()

        # chunk order 0,2,1,3,4: each chunk's AllGather exchange is emitted
        # one stream after its normalization and consumed (by the output
        # projection) later still, hiding the collective latency; only the
        # final 256-col chunk's exchange is tail-serial.
        qorder = [0, 2, 1, 3]
        NE = EH // P  # my output e-tiles (4)
        stream_tasks = {
            0: ([lambda st=st: qn_group(st) for st in range(4, 16)]
                + [lambda hp=hp, sc=sc: k_group(hp, sc)
                   for sc in (1, 2) for hp in range(HP)]
                + [lambda hp=hp: t_group(hp, 2) for hp in range(HP)]),
            2: ([lambda hp=hp: t_group(hp, 1) for hp in range(HP)]
                + [lambda: x_group(0)]),
            1: ([lambda hp=hp: t_group(hp, 3) for hp in range(HP)]
                + [lambda hp=hp: k_group(hp, 3) for hp in range(HP)]
                + [lambda: x_group(2)]
                + [lambda et=et: o_group(0, et) for et in range(NE)]),
            3: ([lambda et=et: o_group(2, et) for et in range(NE)]
                + [lambda: x_group(1)]),
        }

        for cid in qorder:
            j0, w = CHUNKS[cid]
            base, CHc = j0 * P, w * P
            tasks = stream_tasks[cid]
            ti = 0
            t_max = j0 + w - 1
            # dole fill tasks from t=0 so the first units of each head pair
            # (exp-pipeline refill bubbles) stay covered. Safe in every
            # stream: tasks needing the t==2 norm flush of the PREVIOUS
            # stream's last chunk (x_group of it) sit late enough in each
            # list to land after it; o_groups consume chunks flushed two
            # streams earlier.
            tstart = 0
            slots = HP * max(1, t_max + 1 - tstart)
            tspace = max(1, slots // max(1, len(tasks)))
            tper = -(-len(tasks) // slots)  # ceil: tasks per eligible slot
            for hp in range(HP):
                hA, hB = 2 * hp, 2 * hp + 1
                pvA = pav.tile([P, CH], f32, tag="pvA")
                pvB = pav.tile([P, CH], f32, tag="pvB")
                # software pipeline: attnV for unit t is emitted during unit
                # t+1, after the next scores+exp have been issued, so the PE
                # never sits in the scores->exp->attnV dependency chain.
                pipe = None

                def attn_v(ex, t, qoff):
                    nc.tensor.matmul(
                        pvA[0:D + 1, qoff:CHc],
                        qn[:, t, hA * (D + 1):(hA + 1) * (D + 1)],
                        ex[:, 0, qoff:CHc],
                        start=(t == 0),
                        stop=(t == t_max),
                    )
                    nc.tensor.matmul(
                        pvB[0:D + 1, qoff:CHc],
                        qn[:, t, hB * (D + 1):(hB + 1) * (D + 1)],
                        ex[:, 1, qoff:CHc],
                        start=(t == 0),
                        stop=(t == t_max),
                    )

                for t in range(t_max + 1):
                    if t == 2:
                        flush_pending()
                    jloc = max(0, t - j0)
                    qoff = jloc * P
                    sc_t = psc.tile([P, 2, CH], f32, tag="sc_t")
                    nc.tensor.matmul(
                        sc_t[:, 0, qoff:CHc],
                        kT[0:D, hp, t * P:(t + 1) * P],
                        qT[0:D, hp, base + qoff:base + CHc],
                        start=True, stop=True,
                    )
                    nc.tensor.matmul(
                        sc_t[:, 1, qoff:CHc],
                        kT[D:P, hp, t * P:(t + 1) * P],
                        qT[D:P, hp, base + qoff:base + CHc],
                        start=True, stop=True,
                    )
                    ex = expp.tile([P, 2, CH], bf16)
                    nc.scalar.activation(
                        out=ex[:, :, qoff:CHc],
                        in_=sc_t[:, :, qoff:CHc],
                        func=Exp,
                        scale=scale,
                    )
                    if t >= j0:
                        # frontier 128-block: triu multiply (vector, NOT
                        # gpsimd: the collectives block the gpsimd queue)
                        for h2 in range(2):
                            nc.vector.tensor_mul(
                                out=ex[:, h2, qoff:qoff + P],
                                in0=ex[:, h2, qoff:qoff + P],
                                in1=tri,
                            )
                    if pipe is not None:
                        attn_v(*pipe)
                    pipe = (ex, t, qoff)
                    if (t >= tstart and (t - tstart) % tspace == 0
                            and ti < len(tasks)):
                        for _ in range(tper):
                            if ti < len(tasks):
                                tasks[ti]()
                                ti += 1

                attn_v(*pipe)
                # evict unnormalized attn + rowsum row; queue normalization
                stgs = []
                for pv, half in ((pvA, 0), (pvB, 1)):
                    stg = stgp.tile([1, CH], bf16, tag="stg", bufs=2)
                    nc.vector.tensor_copy(out=stg[:, 0:CHc],
                                          in_=pv[D:D + 1, 0:CHc])
                    nc.vector.tensor_copy(
                        out=attnT[half * D:(half + 1) * D, hp,
                                  base:base + CHc],
                        in_=pv[0:D, 0:CHc],
                    )
                    stgs.append(stg)
                pending.append((hp, cid, stgs))
            while ti < len(tasks):
                tasks[ti]()
                ti += 1

        # flush the last pending normalization into spare pav banks, then
        # tail: final 256-col exchange; chunk 3's output projection and warm
        # matmuls fill the PE while the last AllGather is in flight
        wdst2 = psc.tile([P, 2, CH], f32, tag="sc_t")
        for _ in range(4):
            nc.tensor.matmul(wdst2[:, 0, 0:256], wsrc[:, 0:P], wsrc,
                             start=True, stop=True)
        flush_pending(use_pav=True)
        x_group(3)
        for et in range(NE):
            o_group(1, et)
        # 60 warm matmuls: enough to hold the clock at 8/8 through most of
        # the tail AllGather; more (120 tried) overshoots the collective's
        # completion and delays o_group(3) behind the in-order PE queue.
        wdst3 = psc.tile([P, 2, CH], f32, tag="sc_t", name="wdst3")
        for _ in range(60):
            nc.tensor.matmul(wdst3[:, 0, 0:256], wsrc[:, 0:P], wsrc,
                             start=True, stop=True)
        for et in range(NE):
            o_group(3, et)

    nc.finalize()
    return nc


def _prep_inputs(x, Wk, Wq, Wo, bo, n_cores=NCORES):
    """Per-core input maps: batch = c//2, head half = c%2 (all bf16).

    wq/wk columns select the core's 8 heads; wo columns select the core's
    512 OUTPUT dims (e-split outproj over all 16 heads via the exchange).
    """
    b, s, e = x.shape
    P = 128
    EH = e // 2
    wqT = np.ascontiguousarray(Wq.T).astype(BF16)
    wkT = np.ascontiguousarray(Wk.T).astype(BF16)
    woT = np.ascontiguousarray(Wo.T).astype(BF16)
    consts = np.concatenate(
        [np.eye(P, dtype=np.float32),
         np.triu(np.ones((P, P), dtype=np.float32))], axis=1).astype(BF16)
    in_maps = []
    for c in range(n_cores):
        bi, hh = c // 2, c % 2
        xT = np.ascontiguousarray(x[bi].T).astype(BF16)
        bo_col = np.ascontiguousarray(
            bo[hh * EH:(hh + 1) * EH].reshape(EH // P, P).T
        ).astype(np.float32)
        in_maps.append({
            "xT": xT,
            "wqT": np.ascontiguousarray(wqT[:, hh * EH:(hh + 1) * EH]),
            "wkT": np.ascontiguousarray(wkT[:, hh * EH:(hh + 1) * EH]),
            "woT": np.ascontiguousarray(woT[:, hh * EH:(hh + 1) * EH]),
            "bo": bo_col,
            "consts": consts,
        })
    return in_maps


def kernel(x, Wk, Wq, Wv, Wo, bo):
    from concourse import bass_utils

    x = np.asarray(x, dtype=np.float32)
    Wk = np.asarray(Wk, dtype=np.float32)
    Wq = np.asarray(Wq, dtype=np.float32)
    Wo = np.asarray(Wo, dtype=np.float32)
    bo = np.asarray(bo, dtype=np.float32)
    b, s, e = x.shape
    key = (s, e, H)
    if key not in _CACHE:
        _CACHE[key] = _build_program(s, e, H)
    nc = _CACHE[key]
    in_maps = _prep_inputs(x, Wk, Wq, Wo, bo)
    res = bass_utils.run_bass_kernel_spmd(nc, in_maps, list(range(NCORES)))
    out = np.empty((b, s, e), dtype=np.float32)
    EH = e // 2
    for c in range(NCORES):
        bi, hh = c // 2, c % 2
        oc = np.asarray(res.results[c]["out"], dtype=np.float32)  # [EH, S]
        out[bi, :, hh * EH:(hh + 1) * EH] = oc.T
    return out


if __name__ == "__main__":
    nc = _build_program(S, E, H)
    print("built ok")
